# revision 1
# baseline (speedup 1.0000x reference)
"""Trainium2 Bass kernel for nn_GNN_37615323579234 (gnn_message_passing).

Math (reference, N=8192, D=64, 4 layers; layer-3 A@H products are dead code):
    l=0..3:  H_cl = relu(X1@w1+b1) + relu(X2@w2+b2);  H_ue = relu(Xue@w3+b3)
             X1 = A_cl@H_cl;  X2 = A_ue@H_ue;  Xue = A_ue@H_cl
    out = relu(colsum(H_cl3) @ Qw1 + Qb1) @ Qw2 + Qb2      # [1,1]

Sharding: row-shard A_cl/A_ue over 8 cores (1024 rows each).  Host feeds each
core its A row-block TRANSPOSED and cast to bf16 ([8192,1024] contiguous) so
the contraction dim lands on SBUF partitions with line-rate DMA and half the
HBM traffic.  Big matmuls compute the output TRANSPOSED: stationary = H k-tile
(natural layout, bf16), moving = A^T k-tile (bf16), f32 PSUM.  A^T tiles are
DMA'd 4 k-tiles (1 MiB) per transfer.  H_ue|H_cl interleave per k-tile in one
SBUF buffer so the fused A_ue pass uses a single [128,128] stationary.  Biases
fold into the small matmuls via an appended ones-row.  Between layers: an
AllGather of the updated H blocks (DRAM bounce), AllReduce for the pooled vec.
"""

import os
import sys

for _p in ("/opt/trn_rl_repo", "/root/.axon_site/_ro/trn_rl_repo"):
    if os.path.isdir(_p) and _p not in sys.path:
        sys.path.insert(0, _p)

import numpy as np

N = 8192
D = 64
M = 8          # cores
R = N // M     # 1024 rows per core
P = 128        # partitions
KT = N // P    # 64 k-tiles
JT = R // P    # 8 row-tiles per core
KB = 4         # k-tiles per A-stream DMA (1 MiB in bf16)

LAST_EXEC_NS = None
LAST_PROFILE = None

_CACHED = None  # compile once per process


def _build_module():
    import concourse.bacc as bacc
    import concourse.mybir as mybir
    from concourse import tile

    f32 = mybir.dt.float32
    bf16 = mybir.dt.bfloat16
    RELU = mybir.ActivationFunctionType.Relu
    ADD = mybir.AluOpType.add
    BYPASS = mybir.AluOpType.bypass

    nc = bacc.Bacc(
        "TRN2",
        target_bir_lowering=False,
        debug=False,
        enable_asserts=False,
        num_devices=M,
    )

    # ---- I/O -------------------------------------------------------------
    AclT = nc.dram_tensor("AclT", [N, R], bf16, kind="ExternalInput")
    AueT = nc.dram_tensor("AueT", [N, R], bf16, kind="ExternalInput")
    X1T_d = nc.dram_tensor("X1T", [3, N], f32, kind="ExternalInput")
    X2T_d = nc.dram_tensor("X2T", [3, N], f32, kind="ExternalInput")
    XueT_d = nc.dram_tensor("XueT", [3, N], f32, kind="ExternalInput")
    w10_d = nc.dram_tensor("w10", [3, D], f32, kind="ExternalInput")
    w20_d = nc.dram_tensor("w20", [3, D], f32, kind="ExternalInput")
    w30_d = nc.dram_tensor("w30", [3, D], f32, kind="ExternalInput")
    w1x_d = nc.dram_tensor("w1x", [D + 1, 3, D], f32, kind="ExternalInput")
    w2x_d = nc.dram_tensor("w2x", [D + 1, 3, D], f32, kind="ExternalInput")
    w3x_d = nc.dram_tensor("w3x", [D + 1, 3, D], f32, kind="ExternalInput")
    q1x_d = nc.dram_tensor("q1x", [D + 1, D], f32, kind="ExternalInput")
    q2x_d = nc.dram_tensor("q2x", [D + 1, 1], f32, kind="ExternalInput")
    out_d = nc.dram_tensor("out", [1, 1], f32, kind="ExternalOutput")

    # internal DRAM for collectives
    Lg = nc.dram_tensor("Lg", [JT, P, 2 * D], bf16)
    Gg = nc.dram_tensor("Gg", [KT, P, 2 * D], bf16, addr_space="Shared")
    prd_l = nc.dram_tensor("prd_l", [D, 1], f32)
    prd_s = nc.dram_tensor("prd_s", [D, 1], f32, addr_space="Shared")

    groups = [list(range(M))]
    reps = int(os.environ.get("KREPS", "1"))
    nocc = bool(int(os.environ.get("KNOCC", "0")))  # no collectives (timing)
    kmode = os.environ.get("KMODE", "full")         # full | dmaonly

    with tile.TileContext(nc) as tc, tc.tile_pool(name="persist", bufs=1) as pp:
        # persistent SBUF state
        Hbuf = pp.tile([P, KT, 2 * D], bf16, tag="Hbuf")  # [:,k,0:64]=H_ue, 64:128=H_cl
        w10 = pp.tile([3, D], f32, tag="w10s")
        w20 = pp.tile([3, D], f32, tag="w20s")
        w30 = pp.tile([3, D], f32, tag="w30s")
        w1x = pp.tile([D + 1, 3, D], f32, tag="w1xs")
        w2x = pp.tile([D + 1, 3, D], f32, tag="w2xs")
        w3x = pp.tile([D + 1, 3, D], f32, tag="w3xs")
        q1x = pp.tile([D + 1, D], f32, tag="q1xs")
        q2x = pp.tile([D + 1, 1], f32, tag="q2xs")
        ones_mv = pp.tile([P, 1], f32, tag="ones_mv")

        nc.sync.dma_start(out=w10[:], in_=w10_d[:])
        nc.sync.dma_start(out=w20[:], in_=w20_d[:])
        nc.sync.dma_start(out=w30[:], in_=w30_d[:])
        nc.sync.dma_start(out=w1x[:], in_=w1x_d[:])
        nc.sync.dma_start(out=w2x[:], in_=w2x_d[:])
        nc.sync.dma_start(out=w3x[:], in_=w3x_d[:])
        nc.sync.dma_start(out=q1x[:], in_=q1x_d[:])
        nc.sync.dma_start(out=q2x[:], in_=q2x_d[:])
        nc.gpsimd.memset(ones_mv[:], 1.0)

        with (
            tc.tile_pool(name="pa", bufs=5) as pa,
            tc.tile_pool(name="pb", bufs=5) as pb,
            tc.tile_pool(name="ps", bufs=1, space="PSUM") as ps,
            tc.tile_pool(name="sbE", bufs=2) as sbE,
            tc.tile_pool(name="pX", bufs=2) as pX,
        ):
          if kmode != "full":
              nc.gpsimd.memset(Hbuf[:], 0.0)
          for _rep in range(reps):
            # ---- layer 0: full H0 for all N rows, interleaved into Hbuf --
            for g in range(8 if kmode == "full" else 0):
                gsl = slice(g * R, (g + 1) * R)
                x1c = pX.tile([3, R], f32, tag="x1c")
                x2c = pX.tile([3, R], f32, tag="x2c")
                xuc = pX.tile([3, R], f32, tag="xuc")
                nc.sync.dma_start(out=x1c[:], in_=X1T_d[:, gsl])
                nc.sync.dma_start(out=x2c[:], in_=X2T_d[:, gsl])
                nc.sync.dma_start(out=xuc[:], in_=XueT_d[:, gsl])
                pue = ps.tile([P, 8, D], f32, tag="pnue")
                pc1 = ps.tile([P, 8, D], f32, tag="pn1")
                pc2 = ps.tile([P, 8, D], f32, tag="pn2")
                for jj in range(8):
                    sl = slice(jj * P, (jj + 1) * P)
                    nc.tensor.matmul(pue[:, jj, :], xuc[:, sl], w30[:], start=True, stop=True)
                    nc.tensor.matmul(pc1[:, jj, :], x1c[:, sl], w10[:], start=True, stop=True)
                    nc.tensor.matmul(pc2[:, jj, :], x2c[:, sl], w20[:], start=True, stop=True)
                jsl = slice(g * 8, (g + 1) * 8)
                t1 = sbE.tile([P, 8, D], f32, tag="t1")
                t2 = sbE.tile([P, 8, D], f32, tag="t2")
                nc.scalar.activation(Hbuf[:, jsl, 0:D], pue[:], RELU)
                nc.scalar.activation(t1[:], pc1[:], RELU)
                nc.scalar.activation(t2[:], pc2[:], RELU)
                nc.vector.tensor_tensor(Hbuf[:, jsl, D : 2 * D], t1[:], t2[:], ADD)

            # ---- main layers ---------------------------------------------
            for l in range(3):
                last = l == 2
                mue = P if not last else D  # ue-pass stationary width
                Pcl0 = ps.tile([D, 512], f32, tag="acc_cl0")
                Pcl1 = ps.tile([D, 512], f32, tag="acc_cl1")
                Pue0 = ps.tile([mue, 512], f32, tag="acc_ue0")
                Pue1 = ps.tile([mue, 512], f32, tag="acc_ue1")
                for kb in range(KT // KB):
                    rows = slice(kb * KB * P, (kb + 1) * KB * P)
                    at = pa.tile([P, KB, R], bf16, tag="acl")
                    bt = pb.tile([P, KB, R], bf16, tag="aue")
                    nc.sync.dma_start(
                        out=at[:], in_=AclT[rows, :].rearrange("(kk p) r -> p kk r", p=P)
                    )
                    nc.sync.dma_start(
                        out=bt[:], in_=AueT[rows, :].rearrange("(kk p) r -> p kk r", p=P)
                    )
                    for kk in range(KB):
                        k = kb * KB + kk
                        st_cl = Hbuf[:, k, D : 2 * D]
                        st_ue = Hbuf[:, k, 0:mue]
                        s, e = k == 0, k == KT - 1
                        nc.tensor.matmul(Pcl0[:], st_cl, at[:, kk, 0:512], start=s, stop=e)
                        nc.tensor.matmul(Pue0[:], st_ue, bt[:, kk, 0:512], start=s, stop=e)
                        if kmode == "full":
                            nc.tensor.matmul(Pcl1[:], st_cl, at[:, kk, 512:1024], start=s, stop=e)
                            nc.tensor.matmul(Pue1[:], st_ue, bt[:, kk, 512:1024], start=s, stop=e)
                if kmode != "full":
                    continue

                # epilogue: X^T blocks -> next-layer H for this core's rows
                XT1 = sbE.tile([D + 1, R], f32, tag="xt1")
                XT2 = sbE.tile([D + 1, R], f32, tag="xt2")
                nc.vector.tensor_copy(XT1[0:D, 0:512], Pcl0[:])
                nc.vector.tensor_copy(XT1[0:D, 512:1024], Pcl1[:])
                nc.gpsimd.memset(XT1[D : D + 1, :], 1.0)
                nc.vector.tensor_copy(XT2[0:D, 0:512], Pue0[0:D, :])
                nc.vector.tensor_copy(XT2[0:D, 512:1024], Pue1[0:D, :])
                nc.gpsimd.memset(XT2[D : D + 1, :], 1.0)
                if not last:
                    XT3 = sbE.tile([D + 1, R], f32, tag="xt3")
                    nc.vector.tensor_copy(XT3[0:D, 0:512], Pue0[D:P, :])
                    nc.vector.tensor_copy(XT3[0:D, 512:1024], Pue1[D:P, :])
                    nc.gpsimd.memset(XT3[D : D + 1, :], 1.0)

                Pn1 = ps.tile([P, 8, D], f32, tag="pn1")
                Pn2 = ps.tile([P, 8, D], f32, tag="pn2")
                if not last:
                    Pnue = ps.tile([P, 8, D], f32, tag="pnue")
                for jj in range(JT):
                    sl = slice(jj * P, (jj + 1) * P)
                    nc.tensor.matmul(Pn1[:, jj, :], XT1[:, sl], w1x[:, l, :], start=True, stop=True)
                    nc.tensor.matmul(Pn2[:, jj, :], XT2[:, sl], w2x[:, l, :], start=True, stop=True)
                    if not last:
                        nc.tensor.matmul(Pnue[:, jj, :], XT3[:, sl], w3x[:, l, :], start=True, stop=True)

                t1 = sbE.tile([P, 8, D], f32, tag="t1")
                t2 = sbE.tile([P, 8, D], f32, tag="t2")
                nc.scalar.activation(t1[:], Pn1[:], RELU)
                nc.scalar.activation(t2[:], Pn2[:], RELU)

                if not last:
                    Epad = sbE.tile([P, JT, 2 * D], bf16, tag="epad")
                    nc.scalar.activation(Epad[:, :, 0:D], Pnue[:], RELU)
                    nc.vector.tensor_tensor(Epad[:, :, D : 2 * D], t1[:], t2[:], ADD)
                    for jj in range(JT):
                        nc.sync.dma_start(out=Lg[jj], in_=Epad[:, jj, :])
                    if nocc:
                        nc.sync.dma_start(out=Gg[0:JT], in_=Lg[:])
                    else:
                        nc.gpsimd.collective_compute(
                            "AllGather",
                            BYPASS,
                            replica_groups=groups,
                            ins=[Lg[:].opt()],
                            outs=[Gg[:].opt()],
                        )
                    nc.sync.dma_start(
                        out=Hbuf[:], in_=Gg[:].rearrange("j p c -> p j c")
                    )
                else:
                    # H_cl3 block -> column sum -> AllReduce -> head MLP
                    hs = sbE.tile([P, JT, D], f32, tag="hs")
                    nc.vector.tensor_tensor(hs[:], t1[:], t2[:], ADD)
                    Ppool = ps.tile([D, 1], f32, tag="pooled")
                    for jj in range(JT):
                        nc.tensor.matmul(
                            Ppool[:], hs[:, jj, :], ones_mv[:],
                            start=(jj == 0), stop=(jj == JT - 1),
                        )
                    pl_s = sbE.tile([D, 1], f32, tag="pl")
                    nc.vector.tensor_copy(pl_s[:], Ppool[:])
                    nc.sync.dma_start(out=prd_l[:], in_=pl_s[:])
                    if nocc:
                        nc.sync.dma_start(out=prd_s[:], in_=prd_l[:])
                    else:
                        nc.gpsimd.collective_compute(
                            "AllReduce",
                            ADD,
                            replica_groups=groups,
                            ins=[prd_l[:].opt()],
                            outs=[prd_s[:].opt()],
                        )
                    pvec = sbE.tile([D + 1, 1], f32, tag="pvec")
                    nc.sync.dma_start(out=pvec[0:D, :], in_=prd_s[:])
                    nc.gpsimd.memset(pvec[D : D + 1, :], 1.0)
                    Pz = ps.tile([D, 1], f32, tag="pooled")
                    nc.tensor.matmul(Pz[:], q1x[:], pvec[:], start=True, stop=True)
                    zt = sbE.tile([D + 1, 1], f32, tag="zt")
                    nc.scalar.activation(zt[0:D, :], Pz[:], RELU)
                    nc.gpsimd.memset(zt[D : D + 1, :], 1.0)
                    Po = ps.tile([1, 1], f32, tag="pooled")
                    nc.tensor.matmul(Po[:], q2x[:], zt[:], start=True, stop=True)
                    o_s = sbE.tile([1, 1], f32, tag="os")
                    nc.vector.tensor_copy(o_s[:], Po[:])
                    nc.sync.dma_start(out=out_d[:], in_=o_s[:])

            if kmode != "full":
                o_s = sbE.tile([1, 1], f32, tag="os")
                nc.gpsimd.memset(o_s[:], 0.0)
                nc.sync.dma_start(out=out_d[:], in_=o_s[:])

    nc.compile()
    return nc


def _get_module():
    global _CACHED
    if _CACHED is None:
        _CACHED = _build_module()
    return _CACHED


def prep_in_maps(inputs):
    import ml_dtypes

    f = np.float32
    bf = ml_dtypes.bfloat16
    A_cl = np.asarray(inputs["A_cl"], f)
    A_ue = np.asarray(inputs["A_ue"], f)
    ones_row = np.ones((1, N), f)
    X1T = np.ascontiguousarray(np.vstack([np.asarray(inputs["X_cl_1"], f).T, ones_row]))
    X2T = np.ascontiguousarray(np.vstack([np.asarray(inputs["X_cl_2"], f).T, ones_row]))
    XueT = np.ascontiguousarray(np.vstack([np.asarray(inputs["X_ue"], f).T, ones_row]))

    def wx0(w, b):
        return np.ascontiguousarray(np.vstack([np.asarray(w, f), np.asarray(b, f)[None, :]]))

    def wx(w, b):
        # [3, D, D] + [3, D] -> [D+1, 3, D]
        w = np.asarray(w, f)
        b = np.asarray(b, f)
        stk = np.stack([np.vstack([w[i], b[i][None, :]]) for i in range(3)], axis=1)
        return np.ascontiguousarray(stk)

    common = {
        "X1T": X1T,
        "X2T": X2T,
        "XueT": XueT,
        "w10": wx0(inputs["W1_w0"], inputs["W1_b0"]),
        "w20": wx0(inputs["W2_w0"], inputs["W2_b0"]),
        "w30": wx0(inputs["W3_w0"], inputs["W3_b0"]),
        "w1x": wx(inputs["W1_w"], inputs["W1_b"]),
        "w2x": wx(inputs["W2_w"], inputs["W2_b"]),
        "w3x": wx(inputs["W3_w"], inputs["W3_b"]),
        "q1x": wx0(inputs["Q_w1"], inputs["Q_b1"]),
        "q2x": np.ascontiguousarray(
            np.vstack([np.asarray(inputs["Q_w2"], f), np.asarray(inputs["Q_b2"], f)[None, :]])
        ),
    }

    in_maps = []
    for c in range(M):
        rs = slice(c * R, (c + 1) * R)
        m = dict(common)
        m["AclT"] = np.ascontiguousarray(A_cl[rs, :].T.astype(bf))
        m["AueT"] = np.ascontiguousarray(A_ue[rs, :].T.astype(bf))
        in_maps.append(m)
    return in_maps


def kernel(**inputs):
    global LAST_EXEC_NS, LAST_PROFILE
    nc = _get_module()
    from concourse.bass_utils import run_bass_kernel_spmd

    in_maps = prep_in_maps(inputs)
    res = run_bass_kernel_spmd(nc, in_maps, core_ids=list(range(M)), trace=False)
    LAST_EXEC_NS = res.exec_time_ns
    LAST_PROFILE = res.profile_json
    return np.asarray(res.results[0]["out"], np.float32)



# revision 3
# speedup vs baseline: 1.8943x; 1.8943x over previous
"""Trainium2 Bass kernel for nn_GNN_37615323579234 (gnn_message_passing).

Math (reference, N=8192, D=64, 4 layers; layer-3 A@H products are dead code):
    l=0..3:  H_cl = relu(X1@w1+b1) + relu(X2@w2+b2);  H_ue = relu(Xue@w3+b3)
             X1 = A_cl@H_cl;  X2 = A_ue@H_ue;  Xue = A_ue@H_cl
    out = relu(colsum(H_cl3) @ Qw1 + Qb1) @ Qw2 + Qb2      # [1,1]

Strategy: row-shard A_cl/A_ue over 8 cores (1024 rows each).  Host pre-scales
A by 2^13 (entries are uniform[0,1/N]) and casts to fp8 e4m3 so each core's
A^T block pair is 16 MiB — SBUF-RESIDENT.  A is DMA'd from HBM exactly once
(16 x 1 MiB chunks, 8 KiB/partition descriptors) while layer 0 computes; the
layer-1 matmuls consume chunks as they land; layers 2-3 run entirely from
SBUF.  H is stored x2^6 in fp8 e4m3 so the big matmuls run DoubleRow (two
k-tiles per instruction, 2x PE throughput).  All scale factors are powers of
two folded exactly into the f32 weights host-side.  Big matmuls compute X^T:
stationary = H k-tile pair, moving = A^T k-tile pair, f32 PSUM.  H_ue|H_cl
interleave per k-tile so the fused A_ue pass uses one stationary.  Biases
fold in via appended ones-rows.  Between layers: AllGather of H blocks (fp8,
p-major layout so the DRAM bounce uses 1 KiB descriptors), AllReduce for the
pooled vector at the end.
"""

import os
import sys

for _p in ("/opt/trn_rl_repo", "/root/.axon_site/_ro/trn_rl_repo"):
    if os.path.isdir(_p) and _p not in sys.path:
        sys.path.insert(0, _p)

import numpy as np

N = 8192
D = 64
M = 8          # cores
R = N // M     # 1024 rows per core
P = 128        # partitions
KT = N // P    # 64 k-tiles
JT = R // P    # 8 row-tiles per core
KB = 4         # k-tiles per A-load DMA chunk (1 MiB)

SA = 2.0 ** 13  # A storage scale (A entries ~ uniform[0, 1/8192])
SH = 2.0 ** 6   # H storage scale

LAST_EXEC_NS = None
LAST_PROFILE = None

_CACHED = None  # compile once per process


def _build_module():
    import concourse.bacc as bacc
    import concourse.mybir as mybir
    from concourse import tile

    f32 = mybir.dt.float32
    fp8 = mybir.dt.float8e4
    RELU = mybir.ActivationFunctionType.Relu
    ADD = mybir.AluOpType.add
    BYPASS = mybir.AluOpType.bypass
    DR = mybir.MatmulPerfMode.DoubleRow

    nc = bacc.Bacc(
        "TRN2",
        target_bir_lowering=False,
        debug=False,
        enable_asserts=False,
        num_devices=M,
    )

    # ---- I/O -------------------------------------------------------------
    # A^T blocks, fp8, interleaved cl/ue: [p, k, m, r] = A_m[c*R+r, k*P+p]*SA
    Aall_d = nc.dram_tensor("Aall", [P, KT, 2, R], fp8, kind="ExternalInput")
    # layer-0 fused inputs: rows 0-1 X1^T, 2-3 X2^T, 4-5 Xue^T, 6 ones
    Xcat_d = nc.dram_tensor("Xcat", [7, N], f32, kind="ExternalInput")
    # layer-0 fused weights (block-diagonal + bias row), output scale SH
    Wcat_d = nc.dram_tensor("Wcat", [7, 3 * D], f32, kind="ExternalInput")
    w1x_d = nc.dram_tensor("w1x", [D + 1, 3, D], f32, kind="ExternalInput")
    w2x_d = nc.dram_tensor("w2x", [D + 1, 3, D], f32, kind="ExternalInput")
    w3x_d = nc.dram_tensor("w3x", [D + 1, 3, D], f32, kind="ExternalInput")
    q1x_d = nc.dram_tensor("q1x", [D + 1, D], f32, kind="ExternalInput")
    q2x_d = nc.dram_tensor("q2x", [D + 1, 1], f32, kind="ExternalInput")
    out_d = nc.dram_tensor("out", [1, 1], f32, kind="ExternalOutput")

    # internal DRAM for collectives (fp8 H blocks, p-major within each rank)
    Lg = nc.dram_tensor("Lg", [P, JT, 2 * D], fp8)
    Gg = nc.dram_tensor("Gg", [M, P, JT, 2 * D], fp8, addr_space="Shared")
    prd_l = nc.dram_tensor("prd_l", [D, 1], f32)
    prd_s = nc.dram_tensor("prd_s", [D, 1], f32, addr_space="Shared")

    groups = [list(range(M))]
    nocc = bool(int(os.environ.get("KNOCC", "0")))  # no collectives (timing)

    with tile.TileContext(nc) as tc, tc.tile_pool(name="persist", bufs=1) as pp:
        # persistent SBUF state
        Abuf = pp.tile([P, KT, 2, R], fp8, tag="Abuf")    # 128 KiB/partition
        Hbuf = pp.tile([P, KT, 2 * D], fp8, tag="Hbuf")   # [:,k,0:64]=Hue, 64:128=Hcl
        w1x = pp.tile([D + 1, 3, D], f32, tag="w1xs")
        w2x = pp.tile([D + 1, 3, D], f32, tag="w2xs")
        w3x = pp.tile([D + 1, 3, D], f32, tag="w3xs")
        q1x = pp.tile([D + 1, D], f32, tag="q1xs")
        q2x = pp.tile([D + 1, 1], f32, tag="q2xs")
        ones_mv = pp.tile([P, 1], f32, tag="ones_mv")

        nc.sync.dma_start(out=w1x[:], in_=w1x_d[:])
        nc.sync.dma_start(out=w2x[:], in_=w2x_d[:])
        nc.sync.dma_start(out=w3x[:], in_=w3x_d[:])
        nc.sync.dma_start(out=q1x[:], in_=q1x_d[:])
        nc.sync.dma_start(out=q2x[:], in_=q2x_d[:])
        nc.gpsimd.memset(ones_mv[:], 1.0)

        # ---- layer 0 (+ the one-time A load) -----------------------------
        with (
            tc.tile_pool(name="p0", bufs=1) as p0,
            tc.tile_pool(name="p0t", bufs=2) as p0t,
            tc.tile_pool(name="ps0", bufs=2, space="PSUM") as ps0p,
        ):
            Xcat = p0.tile([7, N], f32, tag="xcat")
            Wcat = p0.tile([7, 3 * D], f32, tag="wcat")
            nc.sync.dma_start(out=Xcat[:], in_=Xcat_d[:])
            nc.sync.dma_start(out=Wcat[:], in_=Wcat_d[:])

            # stream the full A block into SBUF (16 x 1 MiB)
            for kb in range(KT // KB):
                ksl = slice(kb * KB, (kb + 1) * KB)
                nc.sync.dma_start(out=Abuf[:, ksl, :, :], in_=Aall_d[:, ksl, :, :])

            # H0 for all N rows (replicated on every core), interleaved fp8
            for b in range(KT // 2):
                ps0 = ps0p.tile([P, 2, 3 * D], f32, tag="ps0")
                for i in range(2):
                    k = 2 * b + i
                    sl = slice(k * P, (k + 1) * P)
                    nc.tensor.matmul(ps0[:, i, :], Xcat[:, sl], Wcat[:],
                                     start=True, stop=True)
                ksl = slice(2 * b, 2 * b + 2)
                t1 = p0t.tile([P, 2, D], f32, tag="t0a")
                t2 = p0t.tile([P, 2, D], f32, tag="t0b")
                nc.scalar.activation(Hbuf[:, ksl, 0:D], ps0[:, :, 2 * D:3 * D], RELU)
                nc.scalar.activation(t1[:], ps0[:, :, 0:D], RELU)
                nc.scalar.activation(t2[:], ps0[:, :, D:2 * D], RELU)
                nc.vector.tensor_tensor(Hbuf[:, ksl, D:2 * D], t1[:], t2[:], ADD)

        # ---- main layers -------------------------------------------------
        with (
            tc.tile_pool(name="sbE", bufs=1) as sbE,
            tc.tile_pool(name="psA", bufs=1, space="PSUM") as psA,
            tc.tile_pool(name="psE", bufs=1, space="PSUM") as psE,
        ):
            for l in range(3):
                last = l == 2
                wue = 2 * D if not last else D  # ue-pass stationary width
                Pcl0 = psA.tile([D, 512], f32, tag="acc_cl0")
                Pcl1 = psA.tile([D, 512], f32, tag="acc_cl1")
                Pue0 = psA.tile([wue, 512], f32, tag="acc_ue0")
                Pue1 = psA.tile([wue, 512], f32, tag="acc_ue1")
                for kp in range(KT // 2):
                    k = 2 * kp
                    ksl = slice(k, k + 2)
                    st_cl = Hbuf[:, ksl, D:2 * D]   # [128,2,64]
                    st_ue = Hbuf[:, ksl, 0:wue]     # [128,2,128] / [128,2,64]
                    s, e = kp == 0, kp == KT // 2 - 1
                    nc.tensor.matmul(Pcl0[:], st_cl, Abuf[:, ksl, 0, 0:512],
                                     start=s, stop=e, perf_mode=DR)
                    nc.tensor.matmul(Pcl1[:], st_cl, Abuf[:, ksl, 0, 512:1024],
                                     start=s, stop=e, perf_mode=DR)
                    nc.tensor.matmul(Pue0[:], st_ue, Abuf[:, ksl, 1, 0:512],
                                     start=s, stop=e, perf_mode=DR)
                    nc.tensor.matmul(Pue1[:], st_ue, Abuf[:, ksl, 1, 512:1024],
                                     start=s, stop=e, perf_mode=DR)

                # epilogue: X^T blocks -> next-layer H for this core's rows
                XT1 = sbE.tile([D + 1, R], f32, tag="xt1")
                XT2 = sbE.tile([D + 1, R], f32, tag="xt2")
                nc.vector.tensor_copy(XT1[0:D, 0:512], Pcl0[:])
                nc.vector.tensor_copy(XT1[0:D, 512:1024], Pcl1[:])
                nc.gpsimd.memset(XT1[D:D + 1, :], 1.0)
                nc.vector.tensor_copy(XT2[0:D, 0:512], Pue0[0:D, :])
                nc.vector.tensor_copy(XT2[0:D, 512:1024], Pue1[0:D, :])
                nc.gpsimd.memset(XT2[D:D + 1, :], 1.0)
                if not last:
                    XT3 = sbE.tile([D + 1, R], f32, tag="xt3")
                    nc.vector.tensor_copy(XT3[0:D, 0:512], Pue0[D:2 * D, :])
                    nc.vector.tensor_copy(XT3[0:D, 512:1024], Pue1[D:2 * D, :])
                    nc.gpsimd.memset(XT3[D:D + 1, :], 1.0)

                Pn1 = psE.tile([P, JT, D], f32, tag="pn1")
                Pn2 = psE.tile([P, JT, D], f32, tag="pn2")
                if not last:
                    Pnue = psE.tile([P, JT, D], f32, tag="pnue")
                for jj in range(JT):
                    sl = slice(jj * P, (jj + 1) * P)
                    nc.tensor.matmul(Pn1[:, jj, :], XT1[:, sl], w1x[:, l, :],
                                     start=True, stop=True)
                    nc.tensor.matmul(Pn2[:, jj, :], XT2[:, sl], w2x[:, l, :],
                                     start=True, stop=True)
                    if not last:
                        nc.tensor.matmul(Pnue[:, jj, :], XT3[:, sl], w3x[:, l, :],
                                         start=True, stop=True)

                t1 = sbE.tile([P, JT, D], f32, tag="t1")
                t2 = sbE.tile([P, JT, D], f32, tag="t2")
                nc.scalar.activation(t1[:], Pn1[:], RELU)
                nc.scalar.activation(t2[:], Pn2[:], RELU)

                if not last:
                    Epad = sbE.tile([P, JT, 2 * D], fp8, tag="epad")
                    nc.scalar.activation(Epad[:, :, 0:D], Pnue[:], RELU)
                    nc.vector.tensor_tensor(Epad[:, :, D:2 * D], t1[:], t2[:], ADD)
                    nc.sync.dma_start(out=Lg[:], in_=Epad[:])
                    if nocc:
                        nc.sync.dma_start(out=Gg[0], in_=Lg[:])
                    else:
                        nc.gpsimd.collective_compute(
                            "AllGather",
                            BYPASS,
                            replica_groups=groups,
                            ins=[Lg[:].opt()],
                            outs=[Gg[:].opt()],
                        )
                    nc.sync.dma_start(
                        out=Hbuf[:].rearrange("p (c j) d -> p c j d", c=M),
                        in_=Gg[:].rearrange("c p j d -> p c j d"),
                    )
                else:
                    # H_cl3 block -> column sum -> AllReduce -> head MLP
                    hs = sbE.tile([P, JT, D], f32, tag="hs")
                    nc.vector.tensor_tensor(hs[:], t1[:], t2[:], ADD)
                    Ppool = psE.tile([D, 1], f32, tag="pooled")
                    for jj in range(JT):
                        nc.tensor.matmul(
                            Ppool[:], hs[:, jj, :], ones_mv[:],
                            start=(jj == 0), stop=(jj == JT - 1),
                        )
                    pl_s = sbE.tile([D, 1], f32, tag="pl")
                    nc.vector.tensor_copy(pl_s[:], Ppool[:])
                    nc.sync.dma_start(out=prd_l[:], in_=pl_s[:])
                    if nocc:
                        nc.sync.dma_start(out=prd_s[:], in_=prd_l[:])
                    else:
                        nc.gpsimd.collective_compute(
                            "AllReduce",
                            ADD,
                            replica_groups=groups,
                            ins=[prd_l[:].opt()],
                            outs=[prd_s[:].opt()],
                        )
                    pvec = sbE.tile([D + 1, 1], f32, tag="pvec")
                    nc.sync.dma_start(out=pvec[0:D, :], in_=prd_s[:])
                    nc.gpsimd.memset(pvec[D:D + 1, :], 1.0)
                    Pz = psE.tile([D, 1], f32, tag="pooled")
                    nc.tensor.matmul(Pz[:], q1x[:], pvec[:], start=True, stop=True)
                    zt = sbE.tile([D + 1, 1], f32, tag="zt")
                    nc.scalar.activation(zt[0:D, :], Pz[:], RELU)
                    nc.gpsimd.memset(zt[D:D + 1, :], 1.0)
                    Po = psE.tile([1, 1], f32, tag="pooled")
                    nc.tensor.matmul(Po[:], q2x[:], zt[:], start=True, stop=True)
                    o_s = sbE.tile([1, 1], f32, tag="os")
                    nc.vector.tensor_copy(o_s[:], Po[:])
                    nc.sync.dma_start(out=out_d[:], in_=o_s[:])

    nc.compile()
    return nc


def _get_module():
    global _CACHED
    if _CACHED is None:
        _CACHED = _build_module()
    return _CACHED


def prep_in_maps(inputs):
    import ml_dtypes

    f = np.float32
    f8 = ml_dtypes.float8_e4m3
    A_cl = np.asarray(inputs["A_cl"], f)
    A_ue = np.asarray(inputs["A_ue"], f)
    ones_row = np.ones((1, N), f)

    Xcat = np.ascontiguousarray(np.vstack([
        np.asarray(inputs["X_cl_1"], f).T,
        np.asarray(inputs["X_cl_2"], f).T,
        np.asarray(inputs["X_ue"], f).T,
        ones_row,
    ]))

    # layer-0 fused block-diagonal weights, output scale SH
    Wcat = np.zeros((7, 3 * D), f)
    Wcat[0:2, 0:D] = np.asarray(inputs["W1_w0"], f) * SH
    Wcat[2:4, D:2 * D] = np.asarray(inputs["W2_w0"], f) * SH
    Wcat[4:6, 2 * D:3 * D] = np.asarray(inputs["W3_w0"], f) * SH
    Wcat[6, 0:D] = np.asarray(inputs["W1_b0"], f) * SH
    Wcat[6, D:2 * D] = np.asarray(inputs["W2_b0"], f) * SH
    Wcat[6, 2 * D:3 * D] = np.asarray(inputs["W3_b0"], f) * SH

    def wx(w, b):
        # [3, D, D] + [3, D] -> [D+1, 3, D]; input X^T carries scale SA*SH,
        # layers 1-2 re-emit H*SH, layer 3 emits unscaled H.
        w = np.asarray(w, f)
        b = np.asarray(b, f)
        cols = []
        for i in range(3):
            w_scale = (1.0 / SA) if i < 2 else (1.0 / (SA * SH))
            b_scale = SH if i < 2 else 1.0
            cols.append(np.vstack([w[i] * w_scale, b[i][None, :] * b_scale]))
        return np.ascontiguousarray(np.stack(cols, axis=1))

    common = {
        "Xcat": Xcat,
        "Wcat": Wcat,
        "w1x": wx(inputs["W1_w"], inputs["W1_b"]),
        "w2x": wx(inputs["W2_w"], inputs["W2_b"]),
        "w3x": wx(inputs["W3_w"], inputs["W3_b"]),
        "q1x": np.ascontiguousarray(
            np.vstack([np.asarray(inputs["Q_w1"], f),
                       np.asarray(inputs["Q_b1"], f)[None, :]])
        ),
        "q2x": np.ascontiguousarray(
            np.vstack([np.asarray(inputs["Q_w2"], f),
                       np.asarray(inputs["Q_b2"], f)[None, :]])
        ),
    }

    # A blocks: [p, k, m, r] = A_m[c*R+r, k*P+p] * SA, fp8
    Acl8 = (A_cl * SA).astype(f8)
    Aue8 = (A_ue * SA).astype(f8)

    in_maps = []
    for c in range(M):
        rs = slice(c * R, (c + 1) * R)
        # [R, N] -> [R, KT, P] -> [P, KT, R]
        acl = Acl8[rs, :].reshape(R, KT, P).transpose(2, 1, 0)
        aue = Aue8[rs, :].reshape(R, KT, P).transpose(2, 1, 0)
        m = dict(common)
        m["Aall"] = np.ascontiguousarray(
            np.stack([acl, aue], axis=2))  # [P, KT, 2, R]
        in_maps.append(m)
    return in_maps


def kernel(**inputs):
    global LAST_EXEC_NS, LAST_PROFILE
    nc = _get_module()
    from concourse.bass_utils import run_bass_kernel_spmd

    in_maps = prep_in_maps(inputs)
    res = run_bass_kernel_spmd(nc, in_maps, core_ids=list(range(M)), trace=False)
    LAST_EXEC_NS = res.exec_time_ns
    LAST_PROFILE = res.profile_json
    return np.asarray(res.results[0]["out"], np.float32)


# revision 10
# speedup vs baseline: 2.2775x; 1.2023x over previous
"""Trainium2 Bass kernel for nn_GNN_37615323579234 (gnn_message_passing).

Math (reference, N=8192, D=64, 4 layers; layer-3 A@H products are dead code):
    l=0..3:  H_cl = relu(X1@w1+b1) + relu(X2@w2+b2);  H_ue = relu(Xue@w3+b3)
             X1 = A_cl@H_cl;  X2 = A_ue@H_ue;  Xue = A_ue@H_cl
    out = relu(colsum(H_cl3) @ Qw1 + Qb1) @ Qw2 + Qb2      # [1,1]

Strategy: row-shard A_cl/A_ue over 8 cores (1024 rows each).  Host pre-scales
A by 2^13 and casts to fp8 e4m3 so each core's A^T block pair is 16 MiB —
SBUF-RESIDENT, DMA'd from HBM exactly once while layer 0 computes.  H is
stored x2^6 in fp8 so the big matmuls run DoubleRow (2 k-tiles / instruction).
All scales are powers of two folded exactly into f32 weights host-side.

Latency structure (the HW findings that shaped it):
- PE HAM clock-gate: the PE runs at 1.2 GHz until ~3.4us of *continuous*
  activity, then 2.4 GHz until a ~3.4us idle gap.  Warm-up matmuls at t=0 and
  keep-warm matmuls across collective gaps keep every real matmul at 2.4 GHz.
- The A load is split by column half: pass A (output rows 0:512) consumes the
  first 8 MiB, finishes ~20us before the full load, and triggers its
  AllGather early; pass B (rows 512:1024) runs from SBUF under AllGather A.
- Inter-layer AllGathers are split in 2 halves (fp8, p-major layout, 1 KiB
  descriptors); each half's ~15us latency is hidden under the other half's
  matmul work in the next layer (alpha = gathered-first k-tiles, beta = rest).
"""

import os
import sys

for _p in ("/opt/trn_rl_repo", "/root/.axon_site/_ro/trn_rl_repo"):
    if os.path.isdir(_p) and _p not in sys.path:
        sys.path.insert(0, _p)

import numpy as np

N = 8192
D = 64
M = 8          # cores
R = N // M     # 1024 rows per core
P = 128        # partitions
KT = N // P    # 64 k-tiles
JT = R // P    # 8 row-tiles per core
KB = 4         # k-tiles per A-load DMA chunk (512 KiB per column half)
HC = 512       # column half width

SA = 2.0 ** 13  # A storage scale (entries ~ uniform[0, 1/8192])
SH = 2.0 ** 6   # H storage scale

NWARM = int(os.environ.get("KWARM", "14"))   # t=0 HAM warm-up matmuls
NKEEP = int(os.environ.get("KKEEP", "8"))    # keep-warm matmuls per gap

LAST_EXEC_NS = None
LAST_PROFILE = None

_CACHED = None  # compile once per process


def _build_module():
    import concourse.bacc as bacc
    import concourse.mybir as mybir
    from concourse import tile

    f32 = mybir.dt.float32
    fp8 = mybir.dt.float8e4
    RELU = mybir.ActivationFunctionType.Relu
    ADD = mybir.AluOpType.add
    BYPASS = mybir.AluOpType.bypass
    DR = mybir.MatmulPerfMode.DoubleRow

    nc = bacc.Bacc(
        "TRN2",
        target_bir_lowering=False,
        debug=False,
        enable_asserts=False,
        num_devices=M,
    )

    # ---- I/O -------------------------------------------------------------
    # A^T blocks, fp8: [p, h, k, m, r'] = A_m[c*R + h*HC + r', k*P+p] * SA
    Aall_d = nc.dram_tensor("Aall", [P, 2, KT, 2, HC], fp8, kind="ExternalInput")
    # layer-0 fused inputs: rows 0-1 X1^T, 2-3 X2^T, 4-5 Xue^T, 6 ones
    Xcat_d = nc.dram_tensor("Xcat", [7, N], f32, kind="ExternalInput")
    # layer-0 fused weights (block-diagonal + bias row), output scale SH
    Wcat_d = nc.dram_tensor("Wcat", [7, 3 * D], f32, kind="ExternalInput")
    w1x_d = nc.dram_tensor("w1x", [D + 1, 3, D], f32, kind="ExternalInput")
    w2x_d = nc.dram_tensor("w2x", [D + 1, 3, D], f32, kind="ExternalInput")
    w3x_d = nc.dram_tensor("w3x", [D + 1, 3, D], f32, kind="ExternalInput")
    q1x_d = nc.dram_tensor("q1x", [D + 1, D], f32, kind="ExternalInput")
    q2x_d = nc.dram_tensor("q2x", [D + 1, 1], f32, kind="ExternalInput")
    out_d = nc.dram_tensor("out", [1, 1], f32, kind="ExternalOutput")

    # internal DRAM for collectives (fp8 H half-blocks, p-major per rank)
    LgA = nc.dram_tensor("LgA", [P, JT // 2, 2 * D], fp8)
    LgB = nc.dram_tensor("LgB", [P, JT // 2, 2 * D], fp8)
    GgA = nc.dram_tensor("GgA", [M, P, JT // 2, 2 * D], fp8, addr_space="Shared")
    GgB = nc.dram_tensor("GgB", [M, P, JT // 2, 2 * D], fp8, addr_space="Shared")
    prd_l = nc.dram_tensor("prd_l", [D, 1], f32)
    prd_s = nc.dram_tensor("prd_s", [D, 1], f32, addr_space="Shared")

    groups = [list(range(M))]
    nocc = bool(int(os.environ.get("KNOCC", "0")))  # no collectives (timing)

    # k-pair start indices by gather half: alpha = j<4 of every core block
    alpha = [c * JT + j for c in range(M) for j in (0, 2)]
    beta = [c * JT + j for c in range(M) for j in (4, 6)]

    def collective(op, alu, ins, outs, nocc_out):
        if nocc:
            nc.sync.dma_start(out=nocc_out, in_=ins)
        else:
            nc.gpsimd.collective_compute(
                op, alu, replica_groups=groups,
                ins=[ins.opt()], outs=[outs.opt()],
            )

    with tile.TileContext(nc) as tc, tc.tile_pool(name="persist", bufs=1) as pp:
        # persistent SBUF state
        Abuf = pp.tile([P, 2, KT, 2, HC], fp8, tag="Abuf")  # 128 KiB/partition
        Hbuf = pp.tile([P, KT, 2 * D], fp8, tag="Hbuf")     # [:,k,0:64]=Hue
        Hb4 = Hbuf[:].rearrange("p (c j) d -> p c j d", c=M)
        w1x = pp.tile([D + 1, 3, D], f32, tag="w1xs")
        w2x = pp.tile([D + 1, 3, D], f32, tag="w2xs")
        w3x = pp.tile([D + 1, 3, D], f32, tag="w3xs")
        q1x = pp.tile([D + 1, D], f32, tag="q1xs")
        q2x = pp.tile([D + 1, 1], f32, tag="q2xs")
        ones_mv = pp.tile([P, 1], f32, tag="ones_mv")
        wscr = pp.tile([P, HC], f32, tag="wscr")  # warm-up operand

        nc.gpsimd.memset(ones_mv[:], 1.0)
        nc.gpsimd.memset(wscr[:], 1.0)

        # ---- phase 0/1: warm-up + A load + layer 0 -----------------------
        with (
            tc.tile_pool(name="p0", bufs=1) as p0,
            tc.tile_pool(name="p0t", bufs=2) as p0t,
            tc.tile_pool(name="ps0", bufs=2, space="PSUM") as ps0p,
            tc.tile_pool(name="psw", bufs=1, space="PSUM") as pswp,
        ):
            Xcat = p0.tile([7, N], f32, tag="xcat")
            Wcat = p0.tile([7, 3 * D], f32, tag="wcat")
            nc.sync.dma_start(out=Xcat[:], in_=Xcat_d[:])
            nc.sync.dma_start(out=Wcat[:], in_=Wcat_d[:])
            nc.sync.dma_start(out=w1x[:], in_=w1x_d[:])
            nc.sync.dma_start(out=w2x[:], in_=w2x_d[:])
            nc.sync.dma_start(out=w3x[:], in_=w3x_d[:])
            nc.sync.dma_start(out=q1x[:], in_=q1x_d[:])
            nc.sync.dma_start(out=q2x[:], in_=q2x_d[:])

            # stream the A block: column half 0 first (feeds pass A)
            for h in range(2):
                for kb in range(KT // KB):
                    ksl = slice(kb * KB, (kb + 1) * KB)
                    nc.sync.dma_start(out=Abuf[:, h, ksl, :, :],
                                      in_=Aall_d[:, h, ksl, :, :])

            # HAM warm-up: ~3.4us of continuous PE work unthrottles the clock
            wps = pswp.tile([P, 3 * D], f32, tag="wps")
            nc.scalar.activation(wscr[:, 0:1], ones_mv[:], RELU)  # ACT table
            for _ in range(NWARM):
                nc.tensor.matmul(wps[:], wscr[:, 0:P], wscr[:, 0:3 * D],
                                 start=True, stop=True)

            # layer 0: H0 for all N rows (replicated on every core), fp8
            for b in range(KT // 2):
                ps0 = ps0p.tile([P, 2, 3 * D], f32, tag="ps0")
                for i in range(2):
                    k = 2 * b + i
                    sl = slice(k * P, (k + 1) * P)
                    nc.tensor.matmul(ps0[:, i, :], Xcat[:, sl], Wcat[:],
                                     start=True, stop=True)
                ksl = slice(2 * b, 2 * b + 2)
                t12 = p0t.tile([P, 2, 2 * D], f32, tag="t12")
                nc.scalar.activation(t12[:], ps0[:, :, 0:2 * D], RELU)
                nc.vector.tensor_scalar_max(Hbuf[:, ksl, 0:D],
                                            ps0[:, :, 2 * D:3 * D], 0.0)
                nc.vector.tensor_tensor(Hbuf[:, ksl, D:2 * D],
                                        t12[:, :, 0:D], t12[:, :, D:2 * D], ADD)

        # ---- main layers -------------------------------------------------
        with (
            tc.tile_pool(name="sbE", bufs=1) as sbE,
            tc.tile_pool(name="psA", bufs=1, space="PSUM") as psA,
            tc.tile_pool(name="psE", bufs=1, space="PSUM") as psE,
        ):
            # epilogue tiles (ones rows written once)
            XT1 = sbE.tile([D + 1, HC], f32, tag="xt1")
            XT2 = sbE.tile([D + 1, HC], f32, tag="xt2")
            XT3 = sbE.tile([D + 1, HC], f32, tag="xt3")
            nc.gpsimd.memset(XT1[D:D + 1, :], 1.0)
            nc.gpsimd.memset(XT2[D:D + 1, :], 1.0)
            nc.gpsimd.memset(XT3[D:D + 1, :], 1.0)

            def keep_warm(n):
                # slow f32 matmuls into a dead PSUM bank bridge idle gaps so
                # HAM doesn't re-throttle; next real use starts start=True.
                pnw = psE.tile([P, JT // 2, D], f32, tag="pn1")
                for _ in range(n):
                    nc.tensor.matmul(pnw[:, 0, :], wscr[:, 0:P],
                                     wscr[:, 0:D], start=True, stop=True)

            def acc_mms(l, pairs, h, Pcl, Pue, s_pairs, e_pairs):
                last = l == 2
                wue = 2 * D if not last else D
                for k0 in pairs:
                    ksl = slice(k0, k0 + 2)
                    s = k0 == s_pairs
                    e = k0 == e_pairs
                    nc.tensor.matmul(Pcl[:], Hbuf[:, ksl, D:2 * D],
                                     Abuf[:, h, ksl, 0, :],
                                     start=s, stop=e, perf_mode=DR)
                    nc.tensor.matmul(Pue[:], Hbuf[:, ksl, 0:wue],
                                     Abuf[:, h, ksl, 1, :],
                                     start=s, stop=e, perf_mode=DR)

            def epilogue_half(l, hf, Pcl, Pue, Ppool=None):
                # hf: 0 = output rows 0:512 (jj 0-3), 1 = rows 512:1024
                last = l == 2
                nc.vector.tensor_copy(XT1[0:D, :], Pcl[:])
                nc.vector.tensor_copy(XT2[0:D, :], Pue[0:D, :])
                if not last:
                    nc.vector.tensor_copy(XT3[0:D, :], Pue[D:2 * D, :])
                Pn1 = psE.tile([P, JT // 2, D], f32, tag="pn1")
                Pn2 = psE.tile([P, JT // 2, D], f32, tag="pn2")
                if not last:
                    Pnue = psE.tile([P, JT // 2, D], f32, tag="pnue")
                for jj in range(JT // 2):
                    sl = slice(jj * P, (jj + 1) * P)
                    nc.tensor.matmul(Pn1[:, jj, :], XT1[:, sl], w1x[:, l, :],
                                     start=True, stop=True)
                    nc.tensor.matmul(Pn2[:, jj, :], XT2[:, sl], w2x[:, l, :],
                                     start=True, stop=True)
                    if not last:
                        nc.tensor.matmul(Pnue[:, jj, :], XT3[:, sl],
                                         w3x[:, l, :], start=True, stop=True)
                t1 = sbE.tile([P, JT // 2, D], f32, tag="t1")
                t2 = sbE.tile([P, JT // 2, D], f32, tag="t2")
                nc.scalar.activation(t1[:], Pn1[:], RELU)
                nc.scalar.activation(t2[:], Pn2[:], RELU)
                if not last:
                    Epad = sbE.tile([P, JT // 2, 2 * D], fp8,
                                    tag=f"epad{hf}")
                    nc.scalar.activation(Epad[:, :, 0:D], Pnue[:], RELU)
                    nc.vector.tensor_tensor(Epad[:, :, D:2 * D],
                                            t1[:], t2[:], ADD)
                    Lg = LgA if hf == 0 else LgB
                    Gg = GgA if hf == 0 else GgB
                    nc.sync.dma_start(out=Lg[:], in_=Epad[:])
                    collective("AllGather", BYPASS, Lg[:], Gg[:], Gg[0])
                else:
                    hs = sbE.tile([P, JT // 2, D], f32, tag="hs")
                    nc.vector.tensor_tensor(hs[:], t1[:], t2[:], ADD)
                    for jj in range(JT // 2):
                        nc.tensor.matmul(
                            Ppool[:], hs[:, jj, :], ones_mv[:],
                            start=(hf == 0 and jj == 0),
                            stop=(hf == 1 and jj == JT // 2 - 1),
                        )

            for l in range(3):
                last = l == 2
                wue = 2 * D if not last else D
                Pcl0 = psA.tile([D, HC], f32, tag="acc_cl0")
                Pcl1 = psA.tile([D, HC], f32, tag="acc_cl1")
                Pue0 = psA.tile([wue, HC], f32, tag="acc_ue0")
                Pue1 = psA.tile([wue, HC], f32, tag="acc_ue1")
                if last:
                    Ppool = psE.tile([D, 1], f32, tag="pooled")
                else:
                    Ppool = None

                if l == 0:
                    # H0 is local; pass A is paced by the arriving A chunks
                    allp = [2 * kp for kp in range(KT // 2)]
                    acc_mms(0, allp, 0, Pcl0, Pue0, 0, KT - 2)
                    epilogue_half(0, 0, Pcl0, Pue0)
                    acc_mms(0, allp, 1, Pcl1, Pue1, 0, KT - 2)
                    epilogue_half(0, 1, Pcl1, Pue1)
                    keep_warm(NKEEP)
                else:
                    # gathered halves arrive as alpha (coll A), beta (coll B)
                    nc.sync.dma_start(
                        out=Hb4[:, :, 0:JT // 2, :],
                        in_=GgA[:].rearrange("c p j d -> p c j d"))
                    nc.sync.dma_start(
                        out=Hb4[:, :, JT // 2:JT, :],
                        in_=GgB[:].rearrange("c p j d -> p c j d"))
                    acc_mms(l, alpha, 0, Pcl0, Pue0, alpha[0], beta[-1])
                    acc_mms(l, alpha, 1, Pcl1, Pue1, alpha[0], beta[-1])
                    acc_mms(l, beta, 0, Pcl0, Pue0, alpha[0], beta[-1])
                    epilogue_half(l, 0, Pcl0, Pue0, Ppool)
                    acc_mms(l, beta, 1, Pcl1, Pue1, alpha[0], beta[-1])
                    epilogue_half(l, 1, Pcl1, Pue1, Ppool)
                    if not last:
                        keep_warm(NKEEP)

            # ---- pooled vector -> AllReduce -> head MLP ------------------
            pl_s = sbE.tile([D, 1], f32, tag="pl")
            nc.vector.tensor_copy(pl_s[:], Ppool[:])
            nc.sync.dma_start(out=prd_l[:], in_=pl_s[:])
            collective("AllReduce", ADD, prd_l[:], prd_s[:], prd_s[:])
            pvec = sbE.tile([D + 1, 1], f32, tag="pvec")
            nc.sync.dma_start(out=pvec[0:D, :], in_=prd_s[:])
            nc.gpsimd.memset(pvec[D:D + 1, :], 1.0)
            Pz = psE.tile([D, 1], f32, tag="pooled")
            nc.tensor.matmul(Pz[:], q1x[:], pvec[:], start=True, stop=True)
            zt = sbE.tile([D + 1, 1], f32, tag="zt")
            nc.scalar.activation(zt[0:D, :], Pz[:], RELU)
            nc.gpsimd.memset(zt[D:D + 1, :], 1.0)
            Po = psE.tile([1, 1], f32, tag="pooled")
            nc.tensor.matmul(Po[:], q2x[:], zt[:], start=True, stop=True)
            o_s = sbE.tile([1, 1], f32, tag="os")
            nc.vector.tensor_copy(o_s[:], Po[:])
            nc.sync.dma_start(out=out_d[:], in_=o_s[:])

    nc.compile()
    return nc


def _get_module():
    global _CACHED
    if _CACHED is None:
        _CACHED = _build_module()
    return _CACHED


def prep_in_maps(inputs):
    import ml_dtypes

    f = np.float32
    f8 = ml_dtypes.float8_e4m3
    A_cl = np.asarray(inputs["A_cl"], f)
    A_ue = np.asarray(inputs["A_ue"], f)
    ones_row = np.ones((1, N), f)

    Xcat = np.ascontiguousarray(np.vstack([
        np.asarray(inputs["X_cl_1"], f).T,
        np.asarray(inputs["X_cl_2"], f).T,
        np.asarray(inputs["X_ue"], f).T,
        ones_row,
    ]))

    # layer-0 fused block-diagonal weights, output scale SH
    Wcat = np.zeros((7, 3 * D), f)
    Wcat[0:2, 0:D] = np.asarray(inputs["W1_w0"], f) * SH
    Wcat[2:4, D:2 * D] = np.asarray(inputs["W2_w0"], f) * SH
    Wcat[4:6, 2 * D:3 * D] = np.asarray(inputs["W3_w0"], f) * SH
    Wcat[6, 0:D] = np.asarray(inputs["W1_b0"], f) * SH
    Wcat[6, D:2 * D] = np.asarray(inputs["W2_b0"], f) * SH
    Wcat[6, 2 * D:3 * D] = np.asarray(inputs["W3_b0"], f) * SH

    def wx(w, b):
        # [3, D, D] + [3, D] -> [D+1, 3, D]; input X^T carries scale SA*SH,
        # layers 1-2 re-emit H*SH, layer 3 emits unscaled H.
        w = np.asarray(w, f)
        b = np.asarray(b, f)
        cols = []
        for i in range(3):
            w_scale = (1.0 / SA) if i < 2 else (1.0 / (SA * SH))
            b_scale = SH if i < 2 else 1.0
            cols.append(np.vstack([w[i] * w_scale, b[i][None, :] * b_scale]))
        return np.ascontiguousarray(np.stack(cols, axis=1))

    common = {
        "Xcat": Xcat,
        "Wcat": Wcat,
        "w1x": wx(inputs["W1_w"], inputs["W1_b"]),
        "w2x": wx(inputs["W2_w"], inputs["W2_b"]),
        "w3x": wx(inputs["W3_w"], inputs["W3_b"]),
        "q1x": np.ascontiguousarray(
            np.vstack([np.asarray(inputs["Q_w1"], f),
                       np.asarray(inputs["Q_b1"], f)[None, :]])
        ),
        "q2x": np.ascontiguousarray(
            np.vstack([np.asarray(inputs["Q_w2"], f),
                       np.asarray(inputs["Q_b2"], f)[None, :]])
        ),
    }

    # A blocks: [p, h, k, m, r'] = A_m[c*R + h*HC + r', k*P + p] * SA, fp8
    Acl8 = (A_cl * SA).astype(f8)
    Aue8 = (A_ue * SA).astype(f8)

    in_maps = []
    for c in range(M):
        rs = slice(c * R, (c + 1) * R)
        # [R, N] -> [h, r', k, p] -> [p, h, k, r']
        acl = Acl8[rs, :].reshape(2, HC, KT, P).transpose(3, 0, 2, 1)
        aue = Aue8[rs, :].reshape(2, HC, KT, P).transpose(3, 0, 2, 1)
        m = dict(common)
        m["Aall"] = np.ascontiguousarray(
            np.stack([acl, aue], axis=3))  # [P, 2, KT, 2, HC]
        in_maps.append(m)
    return in_maps


def kernel(**inputs):
    global LAST_EXEC_NS, LAST_PROFILE
    nc = _get_module()
    from concourse.bass_utils import run_bass_kernel_spmd

    in_maps = prep_in_maps(inputs)
    res = run_bass_kernel_spmd(nc, in_maps, core_ids=list(range(M)), trace=False)
    LAST_EXEC_NS = res.exec_time_ns
    LAST_PROFILE = res.profile_json
    return np.asarray(res.results[0]["out"], np.float32)


# revision 24
# speedup vs baseline: 2.5374x; 1.1141x over previous
"""Trainium2 Bass kernel for nn_GNN_37615323579234 (gnn_message_passing).

Math (reference, N=8192, D=64, 4 layers; layer-3 A@H products are dead code):
    l=0..3:  H_cl = relu(X1@w1+b1) + relu(X2@w2+b2);  H_ue = relu(Xue@w3+b3)
             X1 = A_cl@H_cl;  X2 = A_ue@H_ue;  Xue = A_ue@H_cl
    out = relu(colsum(H_cl3) @ Qw1 + Qb1) @ Qw2 + Qb2      # [1,1]

Strategy: row-shard A_cl/A_ue over 8 cores (1024 rows each).  Host pre-scales
A by 2^13 and casts to fp8 e4m3 so each core's A^T block pair is 16 MiB —
SBUF-RESIDENT, DMA'd from HBM exactly once while layer 0 computes.  H is
stored x2^6 in fp8 so the big matmuls run DoubleRow (2 k-tiles / instruction).
All scales are powers of two folded exactly into f32 weights host-side.

Latency structure (the HW findings that shaped it):
- PE HAM clock-gate: the PE runs at 1.2 GHz until ~3.4us of *continuous*
  activity, then 2.4 GHz until a ~3.4us idle gap.  Warm-up matmuls at t=0 and
  keep-warm matmuls across collective gaps keep every real matmul at 2.4 GHz.
- The A load is split by column half: pass A (output rows 0:512) consumes the
  first 8 MiB, finishes ~20us before the full load, and triggers its
  AllGather early; pass B (rows 512:1024) runs from SBUF under AllGather A.
- Inter-layer AllGathers are split in 2 halves (fp8, p-major layout, 1 KiB
  descriptors); each half's ~15us latency is hidden under the other half's
  matmul work in the next layer (alpha = gathered-first k-tiles, beta = rest).
"""

import os
import sys

for _p in ("/opt/trn_rl_repo", "/root/.axon_site/_ro/trn_rl_repo"):
    if os.path.isdir(_p) and _p not in sys.path:
        sys.path.insert(0, _p)

import numpy as np

N = 8192
D = 64
M = 8          # cores
R = N // M     # 1024 rows per core
P = 128        # partitions
KT = N // P    # 64 k-tiles
JT = R // P    # 8 row-tiles per core
KB = 4         # k-tiles per A-load DMA chunk (512 KiB per column half)
HC = 512       # column half width

SA = 2.0 ** 13  # A storage scale (entries ~ uniform[0, 1/8192])
SH = 2.0 ** 6   # H storage scale

NWARM = int(os.environ.get("KWARM", "28"))   # t=0 HAM warm-up matmuls
NKEEP = int(os.environ.get("KKEEP", "30"))   # keep-warm matmuls per gap

LAST_EXEC_NS = None
LAST_PROFILE = None

_CACHED = None  # compile once per process


def _build_module():
    import concourse.bacc as bacc
    import concourse.mybir as mybir
    from concourse import tile

    f32 = mybir.dt.float32
    bf16 = mybir.dt.bfloat16
    fp8 = mybir.dt.float8e4
    RELU = mybir.ActivationFunctionType.Relu
    ADD = mybir.AluOpType.add
    BYPASS = mybir.AluOpType.bypass
    DR = mybir.MatmulPerfMode.DoubleRow

    nc = bacc.Bacc(
        "TRN2",
        target_bir_lowering=False,
        debug=False,
        enable_asserts=False,
        num_devices=M,
    )

    # ---- I/O -------------------------------------------------------------
    # A^T blocks, fp8: [p, h, k, m, r'] = A_m[c*R + h*HC + r', k*P+p] * SA
    Aall_d = nc.dram_tensor("Aall", [P, 2, KT, 2, HC], fp8, kind="ExternalInput")
    # layer-0 fused inputs: rows 0-1 X1^T, 2-3 X2^T, 4-5 Xue^T, 6 ones
    Xcat_d = nc.dram_tensor("Xcat", [7, N], bf16, kind="ExternalInput")
    # layer-0 fused weights (block-diagonal + bias row), output scale SH
    Wcat_d = nc.dram_tensor("Wcat", [7, 3 * D], bf16, kind="ExternalInput")
    w1x_d = nc.dram_tensor("w1x", [D + 1, 3, D], bf16, kind="ExternalInput")
    w2x_d = nc.dram_tensor("w2x", [D + 1, 3, D], bf16, kind="ExternalInput")
    w3x_d = nc.dram_tensor("w3x", [D + 1, 3, D], bf16, kind="ExternalInput")
    q1x_d = nc.dram_tensor("q1x", [D + 1, D], f32, kind="ExternalInput")
    q2x_d = nc.dram_tensor("q2x", [D + 1, 1], f32, kind="ExternalInput")
    out_d = nc.dram_tensor("out", [1, 1], f32, kind="ExternalOutput")

    # internal DRAM for collectives (fp8 H half-blocks, p-major per rank)
    LgA = nc.dram_tensor("LgA", [P, JT // 2, 2 * D], fp8)
    LgB = nc.dram_tensor("LgB", [P, JT // 2, 2 * D], fp8)
    GgA = nc.dram_tensor("GgA", [M, P, JT // 2, 2 * D], fp8, addr_space="Shared")
    GgB = nc.dram_tensor("GgB", [M, P, JT // 2, 2 * D], fp8, addr_space="Shared")
    prd_l = nc.dram_tensor("prd_l", [D, 1], f32)
    prd_s = nc.dram_tensor("prd_s", [D, 1], f32, addr_space="Shared")
    LgD = nc.dram_tensor("LgD", [1, 4], f32)
    GgD = nc.dram_tensor("GgD", [M, 4], f32, addr_space="Shared")

    groups = [list(range(M))]
    nocc = bool(int(os.environ.get("KNOCC", "0")))  # no collectives (timing)

    # k-pair start indices by gather half: alpha = j<4 of every core block
    alpha = [c * JT + j for c in range(M) for j in (0, 2)]
    beta = [c * JT + j for c in range(M) for j in (4, 6)]

    def collective(op, alu, ins, outs, nocc_out):
        if nocc:
            nc.sync.dma_start(out=nocc_out, in_=ins)
        else:
            nc.gpsimd.collective_compute(
                op, alu, replica_groups=groups,
                ins=[ins.opt()], outs=[outs.opt()],
            )

    with tile.TileContext(nc) as tc, tc.tile_pool(name="persist", bufs=1) as pp:
        # persistent SBUF state
        Abuf = pp.tile([P, 2, KT, 2, HC], fp8, tag="Abuf")  # 128 KiB/partition
        Hbuf = pp.tile([P, KT, 2 * D], fp8, tag="Hbuf")     # [:,k,0:64]=Hue
        Hb4 = Hbuf[:].rearrange("p (c j) d -> p c j d", c=M)
        w1x = pp.tile([D + 1, 3, D], bf16, tag="w1xs")
        w2x = pp.tile([D + 1, 3, D], bf16, tag="w2xs")
        w3x = pp.tile([D + 1, 3, D], bf16, tag="w3xs")
        q1x = pp.tile([D + 1, D], f32, tag="q1xs")
        q2x = pp.tile([D + 1, 1], f32, tag="q2xs")
        ones_mv = pp.tile([P, 1], bf16, tag="ones_mv")
        wscr = pp.tile([P, HC], bf16, tag="wscr")  # warm-up operand

        nc.gpsimd.memset(ones_mv[:], 1.0)
        nc.gpsimd.memset(wscr[:], 1.0)
        # tiny throwaway AllGather: pays the ncfw barrier + first-collective
        # setup cost during phase 1 instead of at the first real gather
        if not nocc:
            nc.gpsimd.collective_compute(
                "AllGather", BYPASS, replica_groups=groups,
                ins=[LgD[:].opt()], outs=[GgD[:].opt()],
            )

        # ---- phase 0/1: warm-up + A load + layer 0 -----------------------
        with (
            tc.tile_pool(name="p0", bufs=1) as p0,
            tc.tile_pool(name="p0t", bufs=2) as p0t,
            tc.tile_pool(name="ps0", bufs=2, space="PSUM") as ps0p,
            tc.tile_pool(name="psw", bufs=1, space="PSUM") as pswp,
        ):
            Xcat = p0.tile([7, N], bf16, tag="xcat")
            Wcat = p0.tile([7, 3 * D], bf16, tag="wcat")
            nc.sync.dma_start(out=Xcat[:], in_=Xcat_d[:])
            nc.sync.dma_start(out=Wcat[:], in_=Wcat_d[:])
            nc.sync.dma_start(out=w1x[:], in_=w1x_d[:])
            nc.sync.dma_start(out=w2x[:], in_=w2x_d[:])
            nc.sync.dma_start(out=w3x[:], in_=w3x_d[:])
            nc.sync.dma_start(out=q1x[:], in_=q1x_d[:])
            nc.sync.dma_start(out=q2x[:], in_=q2x_d[:])

            # stream the A block: column half 0 first (feeds pass A)
            for h in range(2):
                for kb in range(KT // KB):
                    ksl = slice(kb * KB, (kb + 1) * KB)
                    nc.sync.dma_start(out=Abuf[:, h, ksl, :, :],
                                      in_=Aall_d[:, h, ksl, :, :])

            # HAM warm-up: ~3.4us of continuous PE work unthrottles the clock
            wps = pswp.tile([P, 3 * D], f32, tag="wps")
            nc.scalar.activation(wscr[:, 0:1], ones_mv[:], RELU)  # ACT table
            for _ in range(NWARM):
                nc.tensor.matmul(wps[:], wscr[:, 0:P], wscr[:, 0:3 * D],
                                 start=True, stop=True)

            # layer 0: H0 for all N rows (replicated on every core), fp8
            for b in range(KT // 2):
                ps0 = ps0p.tile([P, 2, 3 * D], f32, tag="ps0")
                for i in range(2):
                    k = 2 * b + i
                    sl = slice(k * P, (k + 1) * P)
                    nc.tensor.matmul(ps0[:, i, :], Xcat[:, sl], Wcat[:],
                                     start=True, stop=True)
                ksl = slice(2 * b, 2 * b + 2)
                t12 = p0t.tile([P, 2, 2 * D], bf16, tag="t12")
                nc.scalar.activation(t12[:], ps0[:, :, 0:2 * D], RELU)
                nc.vector.tensor_scalar_max(Hbuf[:, ksl, 0:D],
                                            ps0[:, :, 2 * D:3 * D], 0.0)
                nc.vector.tensor_tensor(Hbuf[:, ksl, D:2 * D],
                                        t12[:, :, 0:D], t12[:, :, D:2 * D], ADD)

        # ---- main layers -------------------------------------------------
        with (
            tc.tile_pool(name="sbE", bufs=1) as sbE,
            tc.tile_pool(name="psA", bufs=1, space="PSUM") as psA,
            tc.tile_pool(name="psE", bufs=1, space="PSUM") as psE,
        ):
            # epilogue tiles (ones rows written once)
            XT1 = sbE.tile([D + 1, HC], bf16, tag="xt1")
            XT2 = sbE.tile([D + 1, HC], bf16, tag="xt2")
            XT3 = sbE.tile([D + 1, HC], bf16, tag="xt3")
            nc.gpsimd.memset(XT1[D:D + 1, :], 1.0)
            nc.gpsimd.memset(XT2[D:D + 1, :], 1.0)
            nc.gpsimd.memset(XT3[D:D + 1, :], 1.0)

            def keep_warm(n):
                # slow f32 matmuls into a dead PSUM bank bridge idle gaps so
                # HAM doesn't re-throttle; next real use starts start=True.
                pnw = psE.tile([P, JT // 2, D], f32, tag="pn1")
                for _ in range(n):
                    nc.tensor.matmul(pnw[:], wscr[:, 0:P],
                                     wscr[:, 0:2 * P], start=True, stop=True)

            def acc_mms(l, pairs, h, Pcl, Pue, s_pairs, e_pairs):
                last = l == 2
                wue = 2 * D if not last else D
                for k0 in pairs:
                    ksl = slice(k0, k0 + 2)
                    s = k0 == s_pairs
                    e = k0 == e_pairs
                    nc.tensor.matmul(Pcl[:], Hbuf[:, ksl, D:2 * D],
                                     Abuf[:, h, ksl, 0, :],
                                     start=s, stop=e, perf_mode=DR)
                    nc.tensor.matmul(Pue[:], Hbuf[:, ksl, 0:wue],
                                     Abuf[:, h, ksl, 1, :],
                                     start=s, stop=e, perf_mode=DR)

            def epilogue_half(l, hf, Pcl, Pue, Ppool=None):
                # hf: 0 = output rows 0:512 (jj 0-3), 1 = rows 512:1024
                last = l == 2
                nc.vector.tensor_copy(XT1[0:D, :], Pcl[:])
                nc.vector.tensor_copy(XT2[0:D, :], Pue[0:D, :])
                if not last:
                    nc.vector.tensor_copy(XT3[0:D, :], Pue[D:2 * D, :])
                Pn1 = psE.tile([P, JT // 2, D], f32, tag="pn1")
                Pn2 = psE.tile([P, JT // 2, D], f32, tag="pn2")
                if not last:
                    Pnue = psE.tile([P, JT // 2, D], f32, tag="pnue")
                for jj in range(JT // 2):
                    sl = slice(jj * P, (jj + 1) * P)
                    nc.tensor.matmul(Pn1[:, jj, :], XT1[:, sl], w1x[:, l, :],
                                     start=True, stop=True)
                    nc.tensor.matmul(Pn2[:, jj, :], XT2[:, sl], w2x[:, l, :],
                                     start=True, stop=True)
                    if not last:
                        nc.tensor.matmul(Pnue[:, jj, :], XT3[:, sl],
                                         w3x[:, l, :], start=True, stop=True)
                t1 = sbE.tile([P, JT // 2, D], f32, tag="t1")
                t2 = sbE.tile([P, JT // 2, D], f32, tag="t2")
                nc.scalar.activation(t1[:], Pn1[:], RELU)
                nc.scalar.activation(t2[:], Pn2[:], RELU)
                if not last:
                    Epad = sbE.tile([P, JT // 2, 2 * D], fp8,
                                    tag=f"epad{hf}")
                    nc.scalar.activation(Epad[:, :, 0:D], Pnue[:], RELU)
                    nc.vector.tensor_tensor(Epad[:, :, D:2 * D],
                                            t1[:], t2[:], ADD)
                    Lg = LgA if hf == 0 else LgB
                    Gg = GgA if hf == 0 else GgB
                    nc.sync.dma_start(out=Lg[:], in_=Epad[:])
                    collective("AllGather", BYPASS, Lg[:], Gg[:], Gg[0])
                else:
                    hs = sbE.tile([P, JT // 2, D], bf16, tag="hs")
                    nc.vector.tensor_tensor(hs[:], t1[:], t2[:], ADD)
                    for jj in range(JT // 2):
                        nc.tensor.matmul(
                            Ppool[:], hs[:, jj, :], ones_mv[:],
                            start=(hf == 0 and jj == 0),
                            stop=(hf == 1 and jj == JT // 2 - 1),
                        )

            for l in range(3):
                last = l == 2
                wue = 2 * D if not last else D
                Pcl0 = psA.tile([D, HC], f32, tag="acc_cl0")
                Pcl1 = psA.tile([D, HC], f32, tag="acc_cl1")
                Pue0 = psA.tile([wue, HC], f32, tag="acc_ue0")
                Pue1 = psA.tile([wue, HC], f32, tag="acc_ue1")
                if last:
                    Ppool = psE.tile([D, 1], f32, tag="pooled")
                else:
                    Ppool = None

                if l == 0:
                    # H0 is local; pass A is paced by the arriving A chunks
                    allp = [2 * kp for kp in range(KT // 2)]
                    acc_mms(0, allp, 0, Pcl0, Pue0, 0, KT - 2)
                    epilogue_half(0, 0, Pcl0, Pue0)
                    acc_mms(0, allp, 1, Pcl1, Pue1, 0, KT - 2)
                    epilogue_half(0, 1, Pcl1, Pue1)
                    keep_warm(NKEEP)
                else:
                    # gathered halves arrive as alpha (coll A), beta (coll B)
                    nc.sync.dma_start(
                        out=Hb4[:, :, 0:JT // 2, :],
                        in_=GgA[:].rearrange("c p j d -> p c j d"))
                    nc.sync.dma_start(
                        out=Hb4[:, :, JT // 2:JT, :],
                        in_=GgB[:].rearrange("c p j d -> p c j d"))
                    acc_mms(l, alpha, 0, Pcl0, Pue0, alpha[0], beta[-1])
                    acc_mms(l, alpha, 1, Pcl1, Pue1, alpha[0], beta[-1])
                    acc_mms(l, beta, 0, Pcl0, Pue0, alpha[0], beta[-1])
                    epilogue_half(l, 0, Pcl0, Pue0, Ppool)
                    acc_mms(l, beta, 1, Pcl1, Pue1, alpha[0], beta[-1])
                    epilogue_half(l, 1, Pcl1, Pue1, Ppool)
                    if not last:
                        keep_warm(NKEEP)

            # ---- pooled vector -> AllReduce -> head MLP ------------------
            pl_s = sbE.tile([D, 1], f32, tag="pl")
            nc.vector.tensor_copy(pl_s[:], Ppool[:])
            nc.sync.dma_start(out=prd_l[:], in_=pl_s[:])
            collective("AllReduce", ADD, prd_l[:], prd_s[:], prd_s[:])
            pvec = sbE.tile([D + 1, 1], f32, tag="pvec")
            zt = sbE.tile([D + 1, 1], f32, tag="zt")
            nc.gpsimd.memset(pvec[D:D + 1, :], 1.0)
            nc.gpsimd.memset(zt[D:D + 1, :], 1.0)
            nc.sync.dma_start(out=pvec[0:D, :], in_=prd_s[:])
            Pz = psE.tile([D, 1], f32, tag="pooled")
            nc.tensor.matmul(Pz[:], q1x[:], pvec[:], start=True, stop=True)
            nc.scalar.activation(zt[0:D, :], Pz[:], RELU)
            Po = psE.tile([1, 1], f32, tag="pooled")
            nc.tensor.matmul(Po[:], q2x[:], zt[:], start=True, stop=True)
            o_s = sbE.tile([1, 1], f32, tag="os")
            nc.vector.tensor_copy(o_s[:], Po[:])
            nc.sync.dma_start(out=out_d[:], in_=o_s[:])

    nc.compile()
    return nc


def _get_module():
    global _CACHED
    if _CACHED is None:
        _CACHED = _build_module()
    return _CACHED


def prep_in_maps(inputs):
    import ml_dtypes

    f = np.float32
    f8 = ml_dtypes.float8_e4m3
    bf = ml_dtypes.bfloat16
    A_cl = np.asarray(inputs["A_cl"], f)
    A_ue = np.asarray(inputs["A_ue"], f)
    ones_row = np.ones((1, N), f)

    Xcat = np.ascontiguousarray(np.vstack([
        np.asarray(inputs["X_cl_1"], f).T,
        np.asarray(inputs["X_cl_2"], f).T,
        np.asarray(inputs["X_ue"], f).T,
        ones_row,
    ]).astype(bf))

    # layer-0 fused block-diagonal weights, output scale SH
    Wcat = np.zeros((7, 3 * D), f)
    Wcat[0:2, 0:D] = np.asarray(inputs["W1_w0"], f) * SH
    Wcat[2:4, D:2 * D] = np.asarray(inputs["W2_w0"], f) * SH
    Wcat[4:6, 2 * D:3 * D] = np.asarray(inputs["W3_w0"], f) * SH
    Wcat[6, 0:D] = np.asarray(inputs["W1_b0"], f) * SH
    Wcat[6, D:2 * D] = np.asarray(inputs["W2_b0"], f) * SH
    Wcat[6, 2 * D:3 * D] = np.asarray(inputs["W3_b0"], f) * SH

    def wx(w, b):
        # [3, D, D] + [3, D] -> [D+1, 3, D]; input X^T carries scale SA*SH,
        # layers 1-2 re-emit H*SH, layer 3 emits unscaled H.
        w = np.asarray(w, f)
        b = np.asarray(b, f)
        cols = []
        for i in range(3):
            w_scale = (1.0 / SA) if i < 2 else (1.0 / (SA * SH))
            b_scale = SH if i < 2 else 1.0
            cols.append(np.vstack([w[i] * w_scale, b[i][None, :] * b_scale]))
        return np.ascontiguousarray(np.stack(cols, axis=1))

    common = {
        "Xcat": Xcat,
        "Wcat": np.ascontiguousarray(Wcat.astype(bf)),
        "w1x": wx(inputs["W1_w"], inputs["W1_b"]).astype(bf),
        "w2x": wx(inputs["W2_w"], inputs["W2_b"]).astype(bf),
        "w3x": wx(inputs["W3_w"], inputs["W3_b"]).astype(bf),
        "q1x": np.ascontiguousarray(
            np.vstack([np.asarray(inputs["Q_w1"], f),
                       np.asarray(inputs["Q_b1"], f)[None, :]])
        ),
        "q2x": np.ascontiguousarray(
            np.vstack([np.asarray(inputs["Q_w2"], f),
                       np.asarray(inputs["Q_b2"], f)[None, :]])
        ),
    }

    # A blocks: [p, h, k, m, r'] = A_m[c*R + h*HC + r', k*P + p] * SA, fp8
    Acl8 = (A_cl * SA).astype(f8)
    Aue8 = (A_ue * SA).astype(f8)

    in_maps = []
    for c in range(M):
        rs = slice(c * R, (c + 1) * R)
        # [R, N] -> [h, r', k, p] -> [p, h, k, r']
        acl = Acl8[rs, :].reshape(2, HC, KT, P).transpose(3, 0, 2, 1)
        aue = Aue8[rs, :].reshape(2, HC, KT, P).transpose(3, 0, 2, 1)
        m = dict(common)
        m["Aall"] = np.ascontiguousarray(
            np.stack([acl, aue], axis=3))  # [P, 2, KT, 2, HC]
        in_maps.append(m)
    return in_maps


def kernel(**inputs):
    global LAST_EXEC_NS, LAST_PROFILE
    nc = _get_module()
    from concourse.bass_utils import run_bass_kernel_spmd

    in_maps = prep_in_maps(inputs)
    res = run_bass_kernel_spmd(nc, in_maps, core_ids=list(range(M)), trace=False)
    LAST_EXEC_NS = res.exec_time_ns
    LAST_PROFILE = res.profile_json
    return np.asarray(res.results[0]["out"], np.float32)


# revision 26
# speedup vs baseline: 2.5726x; 1.0138x over previous
"""Trainium2 Bass kernel for nn_GNN_37615323579234 (gnn_message_passing).

Math (reference, N=8192, D=64, 4 layers; layer-3 A@H products are dead code):
    l=0..3:  H_cl = relu(X1@w1+b1) + relu(X2@w2+b2);  H_ue = relu(Xue@w3+b3)
             X1 = A_cl@H_cl;  X2 = A_ue@H_ue;  Xue = A_ue@H_cl
    out = relu(colsum(H_cl3) @ Qw1 + Qb1) @ Qw2 + Qb2      # [1,1]

Strategy: row-shard A_cl/A_ue over 8 cores (1024 rows each).  Host pre-scales
A by 2^13 and casts to fp8 e4m3 so each core's A^T block pair is 16 MiB —
SBUF-RESIDENT, DMA'd from HBM exactly once while layer 0 computes.  H is
stored x2^6 in fp8 so the big matmuls run DoubleRow (2 k-tiles / instruction).
All scales are powers of two folded exactly into f32 weights host-side.

Latency structure (the HW findings that shaped it):
- PE HAM clock-gate: the PE runs at 1.2 GHz until ~3.4us of *continuous*
  activity, then 2.4 GHz until a ~3.4us idle gap.  Warm-up matmuls at t=0 and
  keep-warm matmuls across collective gaps keep every real matmul at 2.4 GHz.
- The A load is split by column half: pass A (output rows 0:512) consumes the
  first 8 MiB, finishes ~20us before the full load, and triggers its
  AllGather early; pass B (rows 512:1024) runs from SBUF under AllGather A.
- Inter-layer AllGathers are split in 2 halves (fp8, p-major layout, 1 KiB
  descriptors); each half's ~15us latency is hidden under the other half's
  matmul work in the next layer (alpha = gathered-first k-tiles, beta = rest).
"""

import os
import sys

for _p in ("/opt/trn_rl_repo", "/root/.axon_site/_ro/trn_rl_repo"):
    if os.path.isdir(_p) and _p not in sys.path:
        sys.path.insert(0, _p)

import numpy as np

N = 8192
D = 64
M = 8          # cores
R = N // M     # 1024 rows per core
P = 128        # partitions
KT = N // P    # 64 k-tiles
JT = R // P    # 8 row-tiles per core
KB = 4         # k-tiles per A-load DMA chunk (512 KiB per column half)
HC = 512       # column half width

SA = 2.0 ** 13  # A storage scale (entries ~ uniform[0, 1/8192])
SH = 2.0 ** 6   # H storage scale

NWARM = int(os.environ.get("KWARM", "28"))   # t=0 HAM warm-up matmuls
NKEEP = int(os.environ.get("KKEEP", "30"))   # keep-warm matmuls per gap

LAST_EXEC_NS = None
LAST_PROFILE = None

_CACHED = None  # compile once per process


def _build_module():
    import concourse.bacc as bacc
    import concourse.mybir as mybir
    from concourse import tile

    f32 = mybir.dt.float32
    bf16 = mybir.dt.bfloat16
    fp8 = mybir.dt.float8e4
    RELU = mybir.ActivationFunctionType.Relu
    ADD = mybir.AluOpType.add
    BYPASS = mybir.AluOpType.bypass
    DR = mybir.MatmulPerfMode.DoubleRow

    nc = bacc.Bacc(
        "TRN2",
        target_bir_lowering=False,
        debug=False,
        enable_asserts=False,
        num_devices=M,
    )

    # ---- I/O -------------------------------------------------------------
    # A^T blocks, fp8: [p, h, k, m, r'] = A_m[c*R + h*HC + r', k*P+p] * SA
    Aall_d = nc.dram_tensor("Aall", [P, 2, KT, 2, HC], fp8, kind="ExternalInput")
    # layer-0 fused inputs: rows 0-1 X1^T, 2-3 X2^T, 4-5 Xue^T, 6 ones
    Xcat_d = nc.dram_tensor("Xcat", [7, N], bf16, kind="ExternalInput")
    # layer-0 fused weights (block-diagonal + bias row), output scale SH
    Wcat_d = nc.dram_tensor("Wcat", [7, 3 * D], bf16, kind="ExternalInput")
    w1x_d = nc.dram_tensor("w1x", [D + 1, 3, D], bf16, kind="ExternalInput")
    w2x_d = nc.dram_tensor("w2x", [D + 1, 3, D], bf16, kind="ExternalInput")
    w3x_d = nc.dram_tensor("w3x", [D + 1, 3, D], bf16, kind="ExternalInput")
    q1x_d = nc.dram_tensor("q1x", [D + 1, D], f32, kind="ExternalInput")
    q2x_d = nc.dram_tensor("q2x", [D + 1, 1], f32, kind="ExternalInput")
    out_d = nc.dram_tensor("out", [1, 1], f32, kind="ExternalOutput")

    # internal DRAM for collectives (fp8 H half-blocks, p-major per rank)
    LgA = nc.dram_tensor("LgA", [P, JT // 2, 2 * D], fp8)
    LgB = nc.dram_tensor("LgB", [P, JT // 2, 2 * D], fp8)
    GgA = nc.dram_tensor("GgA", [M, P, JT // 2, 2 * D], fp8, addr_space="Shared")
    GgB = nc.dram_tensor("GgB", [M, P, JT // 2, 2 * D], fp8, addr_space="Shared")
    prd_l = nc.dram_tensor("prd_l", [D, 1], f32)
    prd_s = nc.dram_tensor("prd_s", [D, 1], f32, addr_space="Shared")
    LgD = nc.dram_tensor("LgD", [1, 4], f32)
    GgD = nc.dram_tensor("GgD", [M, 4], f32, addr_space="Shared")

    groups = [list(range(M))]
    nocc = bool(int(os.environ.get("KNOCC", "0")))  # no collectives (timing)

    # k-pair start indices by gather half: alpha = j<4 of every core block
    alpha = [c * JT + j for c in range(M) for j in (0, 2)]
    beta = [c * JT + j for c in range(M) for j in (4, 6)]

    def collective(op, alu, ins, outs, nocc_out):
        if nocc:
            nc.sync.dma_start(out=nocc_out, in_=ins)
        else:
            nc.gpsimd.collective_compute(
                op, alu, replica_groups=groups,
                ins=[ins.opt()], outs=[outs.opt()],
            )

    with tile.TileContext(nc) as tc, tc.tile_pool(name="persist", bufs=1) as pp:
        # persistent SBUF state
        Abuf = pp.tile([P, 2, KT, 2, HC], fp8, tag="Abuf")  # 128 KiB/partition
        Hbuf = pp.tile([P, KT, 2 * D], fp8, tag="Hbuf")     # [:,k,0:64]=Hue
        Hb4 = Hbuf[:].rearrange("p (c j) d -> p c j d", c=M)
        w1x = pp.tile([D + 1, 3, D], bf16, tag="w1xs")
        w2x = pp.tile([D + 1, 3, D], bf16, tag="w2xs")
        w3x = pp.tile([D + 1, 3, D], bf16, tag="w3xs")
        q1x = pp.tile([D + 1, D], f32, tag="q1xs")
        q2x = pp.tile([D + 1, 1], f32, tag="q2xs")
        ones_mv = pp.tile([P, 1], bf16, tag="ones_mv")
        wscr = pp.tile([P, HC], bf16, tag="wscr")  # warm-up operand

        nc.gpsimd.memset(ones_mv[:], 1.0)
        nc.gpsimd.memset(wscr[:], 1.0)
        # tiny throwaway AllGather: pays the ncfw barrier + first-collective
        # setup cost during phase 1 instead of at the first real gather
        if not nocc:
            nc.gpsimd.collective_compute(
                "AllGather", BYPASS, replica_groups=groups,
                ins=[LgD[:].opt()], outs=[GgD[:].opt()],
            )

        # ---- phase 0/1: warm-up + A load + layer 0 -----------------------
        with (
            tc.tile_pool(name="p0", bufs=1) as p0,
            tc.tile_pool(name="p0t", bufs=3) as p0t,
            tc.tile_pool(name="ps0", bufs=3, space="PSUM") as ps0p,
            tc.tile_pool(name="psw", bufs=1, space="PSUM") as pswp,
        ):
            Xcat = p0.tile([7, N], bf16, tag="xcat")
            Wcat = p0.tile([7, 3 * D], bf16, tag="wcat")
            nc.sync.dma_start(out=Xcat[:], in_=Xcat_d[:])
            nc.sync.dma_start(out=Wcat[:], in_=Wcat_d[:])
            nc.sync.dma_start(out=w1x[:], in_=w1x_d[:])
            nc.sync.dma_start(out=w2x[:], in_=w2x_d[:])
            nc.sync.dma_start(out=w3x[:], in_=w3x_d[:])
            nc.sync.dma_start(out=q1x[:], in_=q1x_d[:])
            nc.sync.dma_start(out=q2x[:], in_=q2x_d[:])

            # stream the A block: column half 0 first (feeds pass A)
            for h in range(2):
                for kb in range(KT // KB):
                    ksl = slice(kb * KB, (kb + 1) * KB)
                    nc.sync.dma_start(out=Abuf[:, h, ksl, :, :],
                                      in_=Aall_d[:, h, ksl, :, :])

            # HAM warm-up: ~3.4us of continuous PE work unthrottles the clock
            wps = pswp.tile([P, 3 * D], f32, tag="wps")
            nc.scalar.activation(wscr[:, 0:1], ones_mv[:], RELU)  # ACT table
            for _ in range(NWARM):
                nc.tensor.matmul(wps[:], wscr[:, 0:P], wscr[:, 0:3 * D],
                                 start=True, stop=True)

            # layer 0: H0 for all N rows (replicated on every core), fp8
            for b in range(KT // 2):
                ps0 = ps0p.tile([P, 2, 3 * D], f32, tag="ps0")
                for i in range(2):
                    k = 2 * b + i
                    sl = slice(k * P, (k + 1) * P)
                    nc.tensor.matmul(ps0[:, i, :], Xcat[:, sl], Wcat[:],
                                     start=True, stop=True)
                ksl = slice(2 * b, 2 * b + 2)
                t12 = p0t.tile([P, 2, 2 * D], bf16, tag="t12")
                nc.scalar.activation(t12[:], ps0[:, :, 0:2 * D], RELU)
                nc.vector.tensor_scalar_max(Hbuf[:, ksl, 0:D],
                                            ps0[:, :, 2 * D:3 * D], 0.0)
                nc.vector.tensor_tensor(Hbuf[:, ksl, D:2 * D],
                                        t12[:, :, 0:D], t12[:, :, D:2 * D], ADD)

        # ---- main layers -------------------------------------------------
        with (
            tc.tile_pool(name="sbE", bufs=1) as sbE,
            tc.tile_pool(name="psA", bufs=1, space="PSUM") as psA,
            tc.tile_pool(name="psE", bufs=1, space="PSUM") as psE,
        ):
            # epilogue tiles (ones rows written once)
            XT1 = sbE.tile([D + 1, HC], bf16, tag="xt1")
            XT2 = sbE.tile([D + 1, HC], bf16, tag="xt2")
            XT3 = sbE.tile([D + 1, HC], bf16, tag="xt3")
            nc.gpsimd.memset(XT1[D:D + 1, :], 1.0)
            nc.gpsimd.memset(XT2[D:D + 1, :], 1.0)
            nc.gpsimd.memset(XT3[D:D + 1, :], 1.0)

            def keep_warm(n):
                # slow f32 matmuls into a dead PSUM bank bridge idle gaps so
                # HAM doesn't re-throttle; next real use starts start=True.
                pnw = psE.tile([P, JT // 2, D], f32, tag="pn1")
                for _ in range(n):
                    nc.tensor.matmul(pnw[:], wscr[:, 0:P],
                                     wscr[:, 0:2 * P], start=True, stop=True)

            def acc_mms(l, pairs, h, Pcl, Pue, s_pairs, e_pairs):
                last = l == 2
                wue = 2 * D if not last else D
                for k0 in pairs:
                    ksl = slice(k0, k0 + 2)
                    s = k0 == s_pairs
                    e = k0 == e_pairs
                    nc.tensor.matmul(Pcl[:], Hbuf[:, ksl, D:2 * D],
                                     Abuf[:, h, ksl, 0, :],
                                     start=s, stop=e, perf_mode=DR)
                    nc.tensor.matmul(Pue[:], Hbuf[:, ksl, 0:wue],
                                     Abuf[:, h, ksl, 1, :],
                                     start=s, stop=e, perf_mode=DR)

            def epilogue_half(l, hf, Pcl, Pue, Ppool=None):
                # hf: 0 = output rows 0:512 (jj 0-3), 1 = rows 512:1024
                last = l == 2
                nc.vector.tensor_copy(XT1[0:D, :], Pcl[:])
                nc.vector.tensor_copy(XT2[0:D, :], Pue[0:D, :])
                if not last:
                    nc.vector.tensor_copy(XT3[0:D, :], Pue[D:2 * D, :])
                Pn1 = psE.tile([P, JT // 2, D], f32, tag="pn1")
                Pn2 = psE.tile([P, JT // 2, D], f32, tag="pn2")
                if not last:
                    Pnue = psE.tile([P, JT // 2, D], f32, tag="pnue")
                for jj in range(JT // 2):
                    sl = slice(jj * P, (jj + 1) * P)
                    nc.tensor.matmul(Pn1[:, jj, :], XT1[:, sl], w1x[:, l, :],
                                     start=True, stop=True)
                    nc.tensor.matmul(Pn2[:, jj, :], XT2[:, sl], w2x[:, l, :],
                                     start=True, stop=True)
                    if not last:
                        nc.tensor.matmul(Pnue[:, jj, :], XT3[:, sl],
                                         w3x[:, l, :], start=True, stop=True)
                t1 = sbE.tile([P, JT // 2, D], f32, tag="t1")
                t2 = sbE.tile([P, JT // 2, D], f32, tag="t2")
                nc.scalar.activation(t1[:], Pn1[:], RELU)
                nc.scalar.activation(t2[:], Pn2[:], RELU)
                if not last:
                    Epad = sbE.tile([P, JT // 2, 2 * D], fp8,
                                    tag=f"epad{hf}")
                    nc.scalar.activation(Epad[:, :, 0:D], Pnue[:], RELU)
                    nc.vector.tensor_tensor(Epad[:, :, D:2 * D],
                                            t1[:], t2[:], ADD)
                    Lg = LgA if hf == 0 else LgB
                    Gg = GgA if hf == 0 else GgB
                    nc.sync.dma_start(out=Lg[:], in_=Epad[:])
                    collective("AllGather", BYPASS, Lg[:], Gg[:], Gg[0])
                else:
                    hs = sbE.tile([P, JT // 2, D], bf16, tag="hs")
                    nc.vector.tensor_tensor(hs[:], t1[:], t2[:], ADD)
                    for jj in range(JT // 2):
                        nc.tensor.matmul(
                            Ppool[:], hs[:, jj, :], ones_mv[:],
                            start=(hf == 0 and jj == 0),
                            stop=(hf == 1 and jj == JT // 2 - 1),
                        )

            for l in range(3):
                last = l == 2
                wue = 2 * D if not last else D
                Pcl0 = psA.tile([D, HC], f32, tag="acc_cl0")
                Pcl1 = psA.tile([D, HC], f32, tag="acc_cl1")
                Pue0 = psA.tile([wue, HC], f32, tag="acc_ue0")
                Pue1 = psA.tile([wue, HC], f32, tag="acc_ue1")
                if last:
                    Ppool = psE.tile([D, 1], f32, tag="pooled")
                else:
                    Ppool = None

                if l == 0:
                    # H0 is local; pass A is paced by the arriving A chunks
                    allp = [2 * kp for kp in range(KT // 2)]
                    acc_mms(0, allp, 0, Pcl0, Pue0, 0, KT - 2)
                    epilogue_half(0, 0, Pcl0, Pue0)
                    acc_mms(0, allp, 1, Pcl1, Pue1, 0, KT - 2)
                    epilogue_half(0, 1, Pcl1, Pue1)
                    keep_warm(NKEEP)
                else:
                    # gathered halves arrive as alpha (coll A), beta (coll B)
                    nc.sync.dma_start(
                        out=Hb4[:, :, 0:JT // 2, :],
                        in_=GgA[:].rearrange("c p j d -> p c j d"))
                    nc.sync.dma_start(
                        out=Hb4[:, :, JT // 2:JT, :],
                        in_=GgB[:].rearrange("c p j d -> p c j d"))
                    acc_mms(l, alpha, 0, Pcl0, Pue0, alpha[0], beta[-1])
                    acc_mms(l, beta, 0, Pcl0, Pue0, alpha[0], beta[-1])
                    epilogue_half(l, 0, Pcl0, Pue0, Ppool)
                    acc_mms(l, alpha, 1, Pcl1, Pue1, alpha[0], beta[-1])
                    acc_mms(l, beta, 1, Pcl1, Pue1, alpha[0], beta[-1])
                    epilogue_half(l, 1, Pcl1, Pue1, Ppool)
                    if not last:
                        keep_warm(NKEEP)

            # ---- pooled vector -> AllReduce -> head MLP ------------------
            pl_s = sbE.tile([D, 1], f32, tag="pl")
            nc.vector.tensor_copy(pl_s[:], Ppool[:])
            nc.sync.dma_start(out=prd_l[:], in_=pl_s[:])
            collective("AllReduce", ADD, prd_l[:], prd_s[:], prd_s[:])
            pvec = sbE.tile([D + 1, 1], f32, tag="pvec")
            zt = sbE.tile([D + 1, 1], f32, tag="zt")
            nc.gpsimd.memset(pvec[D:D + 1, :], 1.0)
            nc.gpsimd.memset(zt[D:D + 1, :], 1.0)
            nc.sync.dma_start(out=pvec[0:D, :], in_=prd_s[:])
            Pz = psE.tile([D, 1], f32, tag="pooled")
            nc.tensor.matmul(Pz[:], q1x[:], pvec[:], start=True, stop=True)
            nc.scalar.activation(zt[0:D, :], Pz[:], RELU)
            Po = psE.tile([1, 1], f32, tag="pooled")
            nc.tensor.matmul(Po[:], q2x[:], zt[:], start=True, stop=True)
            o_s = sbE.tile([1, 1], f32, tag="os")
            nc.vector.tensor_copy(o_s[:], Po[:])
            nc.sync.dma_start(out=out_d[:], in_=o_s[:])

    nc.compile()
    return nc


def _get_module():
    global _CACHED
    if _CACHED is None:
        _CACHED = _build_module()
    return _CACHED


def prep_in_maps(inputs):
    import ml_dtypes

    f = np.float32
    f8 = ml_dtypes.float8_e4m3
    bf = ml_dtypes.bfloat16
    A_cl = np.asarray(inputs["A_cl"], f)
    A_ue = np.asarray(inputs["A_ue"], f)
    ones_row = np.ones((1, N), f)

    Xcat = np.ascontiguousarray(np.vstack([
        np.asarray(inputs["X_cl_1"], f).T,
        np.asarray(inputs["X_cl_2"], f).T,
        np.asarray(inputs["X_ue"], f).T,
        ones_row,
    ]).astype(bf))

    # layer-0 fused block-diagonal weights, output scale SH
    Wcat = np.zeros((7, 3 * D), f)
    Wcat[0:2, 0:D] = np.asarray(inputs["W1_w0"], f) * SH
    Wcat[2:4, D:2 * D] = np.asarray(inputs["W2_w0"], f) * SH
    Wcat[4:6, 2 * D:3 * D] = np.asarray(inputs["W3_w0"], f) * SH
    Wcat[6, 0:D] = np.asarray(inputs["W1_b0"], f) * SH
    Wcat[6, D:2 * D] = np.asarray(inputs["W2_b0"], f) * SH
    Wcat[6, 2 * D:3 * D] = np.asarray(inputs["W3_b0"], f) * SH

    def wx(w, b):
        # [3, D, D] + [3, D] -> [D+1, 3, D]; input X^T carries scale SA*SH,
        # layers 1-2 re-emit H*SH, layer 3 emits unscaled H.
        w = np.asarray(w, f)
        b = np.asarray(b, f)
        cols = []
        for i in range(3):
            w_scale = (1.0 / SA) if i < 2 else (1.0 / (SA * SH))
            b_scale = SH if i < 2 else 1.0
            cols.append(np.vstack([w[i] * w_scale, b[i][None, :] * b_scale]))
        return np.ascontiguousarray(np.stack(cols, axis=1))

    common = {
        "Xcat": Xcat,
        "Wcat": np.ascontiguousarray(Wcat.astype(bf)),
        "w1x": wx(inputs["W1_w"], inputs["W1_b"]).astype(bf),
        "w2x": wx(inputs["W2_w"], inputs["W2_b"]).astype(bf),
        "w3x": wx(inputs["W3_w"], inputs["W3_b"]).astype(bf),
        "q1x": np.ascontiguousarray(
            np.vstack([np.asarray(inputs["Q_w1"], f),
                       np.asarray(inputs["Q_b1"], f)[None, :]])
        ),
        "q2x": np.ascontiguousarray(
            np.vstack([np.asarray(inputs["Q_w2"], f),
                       np.asarray(inputs["Q_b2"], f)[None, :]])
        ),
    }

    # A blocks: [p, h, k, m, r'] = A_m[c*R + h*HC + r', k*P + p] * SA, fp8
    Acl8 = (A_cl * SA).astype(f8)
    Aue8 = (A_ue * SA).astype(f8)

    in_maps = []
    for c in range(M):
        rs = slice(c * R, (c + 1) * R)
        # [R, N] -> [h, r', k, p] -> [p, h, k, r']
        acl = Acl8[rs, :].reshape(2, HC, KT, P).transpose(3, 0, 2, 1)
        aue = Aue8[rs, :].reshape(2, HC, KT, P).transpose(3, 0, 2, 1)
        m = dict(common)
        m["Aall"] = np.ascontiguousarray(
            np.stack([acl, aue], axis=3))  # [P, 2, KT, 2, HC]
        in_maps.append(m)
    return in_maps


def kernel(**inputs):
    global LAST_EXEC_NS, LAST_PROFILE
    nc = _get_module()
    from concourse.bass_utils import run_bass_kernel_spmd

    in_maps = prep_in_maps(inputs)
    res = run_bass_kernel_spmd(nc, in_maps, core_ids=list(range(M)), trace=False)
    LAST_EXEC_NS = res.exec_time_ns
    LAST_PROFILE = res.profile_json
    return np.asarray(res.results[0]["out"], np.float32)


# revision 28
# speedup vs baseline: 2.7473x; 1.0679x over previous
"""Trainium2 Bass kernel for nn_GNN_37615323579234 (gnn_message_passing).

Math (reference, N=8192, D=64, 4 layers; layer-3 A@H products are dead code):
    l=0..3:  H_cl = relu(X1@w1+b1) + relu(X2@w2+b2);  H_ue = relu(Xue@w3+b3)
             X1 = A_cl@H_cl;  X2 = A_ue@H_ue;  Xue = A_ue@H_cl
    out = relu(colsum(H_cl3) @ Qw1 + Qb1) @ Qw2 + Qb2      # [1,1]

Strategy: row-shard A_cl/A_ue over 8 cores (1024 rows each).  Host pre-scales
A by 2^13 and casts to fp8 e4m3 so each core's A^T block pair is 16 MiB —
SBUF-RESIDENT, DMA'd from HBM exactly once while layer 0 computes.  H is
stored x2^6 in fp8 so the big matmuls run DoubleRow (2 k-tiles / instruction).
All scales are powers of two folded exactly into f32 weights host-side.

Latency structure (the HW findings that shaped it):
- PE HAM clock-gate: the PE runs at 1.2 GHz until ~3.4us of *continuous*
  activity, then 2.4 GHz until a ~3.4us idle gap.  Warm-up matmuls at t=0 and
  keep-warm matmuls across collective gaps keep every real matmul at 2.4 GHz.
- The A load is split by column half: pass A (output rows 0:512) consumes the
  first 8 MiB, finishes ~20us before the full load, and triggers its
  AllGather early; pass B (rows 512:1024) runs from SBUF under AllGather A.
- Inter-layer AllGathers are split in 2 halves (fp8, p-major layout, 1 KiB
  descriptors); each half's ~15us latency is hidden under the other half's
  matmul work in the next layer (alpha = gathered-first k-tiles, beta = rest).
"""

import os
import sys

for _p in ("/opt/trn_rl_repo", "/root/.axon_site/_ro/trn_rl_repo"):
    if os.path.isdir(_p) and _p not in sys.path:
        sys.path.insert(0, _p)

import numpy as np

N = 8192
D = 64
M = 8          # cores
R = N // M     # 1024 rows per core
P = 128        # partitions
KT = N // P    # 64 k-tiles
JT = R // P    # 8 row-tiles per core
KB = 4         # k-tiles per A-load DMA chunk (512 KiB per column half)
HC = 512       # column half width

SA = 2.0 ** 13  # A storage scale (entries ~ uniform[0, 1/8192])
SH = 2.0 ** 6   # H storage scale

NWARM = int(os.environ.get("KWARM", "28"))   # t=0 HAM warm-up matmuls
NKEEP = int(os.environ.get("KKEEP", "30"))   # keep-warm matmuls per gap

LAST_EXEC_NS = None
LAST_PROFILE = None

_CACHED = None  # compile once per process


def _build_module():
    import concourse.bacc as bacc
    import concourse.mybir as mybir
    from concourse import tile

    f32 = mybir.dt.float32
    bf16 = mybir.dt.bfloat16
    fp8 = mybir.dt.float8e4
    RELU = mybir.ActivationFunctionType.Relu
    ADD = mybir.AluOpType.add
    BYPASS = mybir.AluOpType.bypass
    DR = mybir.MatmulPerfMode.DoubleRow

    nc = bacc.Bacc(
        "TRN2",
        target_bir_lowering=False,
        debug=False,
        enable_asserts=False,
        num_devices=M,
    )

    # ---- I/O -------------------------------------------------------------
    # A^T blocks, fp8: [p, h, k, m, r'] = A_m[c*R + h*HC + r', k*P+p] * SA
    Aall_d = nc.dram_tensor("Aall", [P, 2, KT, 2, HC], fp8, kind="ExternalInput")
    # layer-0 fused inputs: rows 0-1 X1^T, 2-3 X2^T, 4-5 Xue^T, 6 ones
    Xcat_d = nc.dram_tensor("Xcat", [7, N], bf16, kind="ExternalInput")
    # layer-0 fused weights (block-diagonal + bias row), output scale SH
    Wcat_d = nc.dram_tensor("Wcat", [7, 3 * D], bf16, kind="ExternalInput")
    w1x_d = nc.dram_tensor("w1x", [D + 1, 3, D], bf16, kind="ExternalInput")
    w2x_d = nc.dram_tensor("w2x", [D + 1, 3, D], bf16, kind="ExternalInput")
    w3x_d = nc.dram_tensor("w3x", [D + 1, 3, D], bf16, kind="ExternalInput")
    q1x_d = nc.dram_tensor("q1x", [D + 1, D], f32, kind="ExternalInput")
    q2x_d = nc.dram_tensor("q2x", [D + 1, 1], f32, kind="ExternalInput")
    out_d = nc.dram_tensor("out", [1, 1], f32, kind="ExternalOutput")

    # internal DRAM for collectives (fp8 H half-blocks, p-major per rank)
    LgA = nc.dram_tensor("LgA", [P, JT // 2, 2 * D], fp8)
    LgB = nc.dram_tensor("LgB", [P, JT // 2, 2 * D], fp8)
    GgA = nc.dram_tensor("GgA", [M, P, JT // 2, 2 * D], fp8, addr_space="Shared")
    GgB = nc.dram_tensor("GgB", [M, P, JT // 2, 2 * D], fp8, addr_space="Shared")
    prd_l = nc.dram_tensor("prd_l", [D, 1], f32)
    prd_s = nc.dram_tensor("prd_s", [D, 1], f32, addr_space="Shared")
    LgD = nc.dram_tensor("LgD", [1, 4], f32)
    GgD = nc.dram_tensor("GgD", [M, 4], f32, addr_space="Shared")

    groups = [list(range(M))]
    nocc = bool(int(os.environ.get("KNOCC", "0")))  # no collectives (timing)

    # k-pair start indices by gather half: alpha = j<4 of every core block
    alpha = [c * JT + j for c in range(M) for j in (0, 2)]
    beta = [c * JT + j for c in range(M) for j in (4, 6)]

    def collective(op, alu, ins, outs, nocc_out):
        if nocc:
            nc.sync.dma_start(out=nocc_out, in_=ins)
        else:
            nc.gpsimd.collective_compute(
                op, alu, replica_groups=groups,
                ins=[ins.opt()], outs=[outs.opt()],
            )

    with tile.TileContext(nc) as tc, tc.tile_pool(name="persist", bufs=1) as pp:
        # persistent SBUF state
        Abuf = pp.tile([P, 2, KT, 2, HC], fp8, tag="Abuf")  # 128 KiB/partition
        Hbuf = pp.tile([P, KT, 2 * D], fp8, tag="Hbuf")     # [:,k,0:64]=Hue
        Hb4 = Hbuf[:].rearrange("p (c j) d -> p c j d", c=M)
        w1x = pp.tile([D + 1, 3, D], bf16, tag="w1xs")
        w2x = pp.tile([D + 1, 3, D], bf16, tag="w2xs")
        w3x = pp.tile([D + 1, 3, D], bf16, tag="w3xs")
        q1x = pp.tile([D + 1, D], f32, tag="q1xs")
        q2x = pp.tile([D + 1, 1], f32, tag="q2xs")
        ones_mv = pp.tile([P, 1], bf16, tag="ones_mv")
        wscr = pp.tile([P, HC], bf16, tag="wscr")  # warm-up operand
        # epilogue X^T staging (ones rows written once, before the dummy
        # collective trigger occupies the gpsimd queue)
        XT1 = pp.tile([D + 1, HC], bf16, tag="xt1")
        XT2 = pp.tile([D + 1, HC], bf16, tag="xt2")
        XT3 = pp.tile([D + 1, HC], bf16, tag="xt3")

        nc.gpsimd.memset(ones_mv[:], 1.0)
        nc.gpsimd.memset(wscr[:], 1.0)
        nc.gpsimd.memset(XT1[D:D + 1, :], 1.0)
        nc.gpsimd.memset(XT2[D:D + 1, :], 1.0)
        nc.gpsimd.memset(XT3[D:D + 1, :], 1.0)
        # tiny throwaway AllGather: pays the ncfw barrier + first-collective
        # setup cost during phase 1 instead of at the first real gather
        if not nocc:
            nc.gpsimd.collective_compute(
                "AllGather", BYPASS, replica_groups=groups,
                ins=[LgD[:].opt()], outs=[GgD[:].opt()],
            )

        # ---- phase 0/1: warm-up + A load + layer 0 -----------------------
        with (
            tc.tile_pool(name="p0", bufs=1) as p0,
            tc.tile_pool(name="p0t", bufs=3) as p0t,
            tc.tile_pool(name="ps0", bufs=3, space="PSUM") as ps0p,
            tc.tile_pool(name="psw", bufs=1, space="PSUM") as pswp,
        ):
            Xcat = p0.tile([7, N], bf16, tag="xcat")
            Wcat = p0.tile([7, 3 * D], bf16, tag="wcat")
            nc.sync.dma_start(out=Xcat[:], in_=Xcat_d[:])
            nc.sync.dma_start(out=Wcat[:], in_=Wcat_d[:])
            nc.sync.dma_start(out=w1x[:], in_=w1x_d[:])
            nc.sync.dma_start(out=w2x[:], in_=w2x_d[:])
            nc.sync.dma_start(out=w3x[:], in_=w3x_d[:])
            nc.sync.dma_start(out=q1x[:], in_=q1x_d[:])
            nc.sync.dma_start(out=q2x[:], in_=q2x_d[:])

            # stream the A block: column half 0 first (feeds pass A)
            for h in range(2):
                for kb in range(KT // KB):
                    ksl = slice(kb * KB, (kb + 1) * KB)
                    nc.sync.dma_start(out=Abuf[:, h, ksl, :, :],
                                      in_=Aall_d[:, h, ksl, :, :])

            # HAM warm-up: ~3.4us of continuous PE work unthrottles the clock
            wps = pswp.tile([P, 3 * D], f32, tag="wps")
            nc.scalar.activation(wscr[:, 0:1], ones_mv[:], RELU)  # ACT table
            for _ in range(NWARM):
                nc.tensor.matmul(wps[:], wscr[:, 0:P], wscr[:, 0:3 * D],
                                 start=True, stop=True)

            # layer 0: H0 for all N rows (replicated on every core), fp8
            for b in range(KT // 2):
                ps0 = ps0p.tile([P, 2, 3 * D], f32, tag="ps0")
                for i in range(2):
                    k = 2 * b + i
                    sl = slice(k * P, (k + 1) * P)
                    nc.tensor.matmul(ps0[:, i, :], Xcat[:, sl], Wcat[:],
                                     start=True, stop=True)
                ksl = slice(2 * b, 2 * b + 2)
                t12 = p0t.tile([P, 2, 2 * D], bf16, tag="t12")
                nc.scalar.activation(t12[:], ps0[:, :, 0:2 * D], RELU)
                nc.vector.tensor_scalar_max(Hbuf[:, ksl, 0:D],
                                            ps0[:, :, 2 * D:3 * D], 0.0)
                nc.vector.tensor_tensor(Hbuf[:, ksl, D:2 * D],
                                        t12[:, :, 0:D], t12[:, :, D:2 * D], ADD)

        # ---- main layers -------------------------------------------------
        with (
            tc.tile_pool(name="sbE", bufs=1) as sbE,
            tc.tile_pool(name="psA", bufs=1, space="PSUM") as psA,
            tc.tile_pool(name="psE", bufs=1, space="PSUM") as psE,
        ):
            def keep_warm(n):
                # slow f32 matmuls into a dead PSUM bank bridge idle gaps so
                # HAM doesn't re-throttle; next real use starts start=True.
                pnw = psE.tile([P, JT // 2, D], f32, tag="pn1")
                for _ in range(n):
                    nc.tensor.matmul(pnw[:], wscr[:, 0:P],
                                     wscr[:, 0:2 * P], start=True, stop=True)

            def acc_mms(l, pairs, h, Pcl, Pue, s_pairs, e_pairs):
                last = l == 2
                wue = 2 * D if not last else D
                for k0 in pairs:
                    ksl = slice(k0, k0 + 2)
                    s = k0 == s_pairs
                    e = k0 == e_pairs
                    nc.tensor.matmul(Pcl[:], Hbuf[:, ksl, D:2 * D],
                                     Abuf[:, h, ksl, 0, :],
                                     start=s, stop=e, perf_mode=DR)
                    nc.tensor.matmul(Pue[:], Hbuf[:, ksl, 0:wue],
                                     Abuf[:, h, ksl, 1, :],
                                     start=s, stop=e, perf_mode=DR)

            def epilogue_half(l, hf, Pcl, Pue, Ppool=None):
                # hf: 0 = output rows 0:512 (jj 0-3), 1 = rows 512:1024
                last = l == 2
                nc.vector.tensor_copy(XT1[0:D, :], Pcl[:])
                nc.vector.tensor_copy(XT2[0:D, :], Pue[0:D, :])
                if not last:
                    nc.vector.tensor_copy(XT3[0:D, :], Pue[D:2 * D, :])
                Pn1 = psE.tile([P, JT // 2, D], f32, tag="pn1")
                Pn2 = psE.tile([P, JT // 2, D], f32, tag="pn2")
                if not last:
                    Pnue = psE.tile([P, JT // 2, D], f32, tag="pnue")
                for jj in range(JT // 2):
                    sl = slice(jj * P, (jj + 1) * P)
                    nc.tensor.matmul(Pn1[:, jj, :], XT1[:, sl], w1x[:, l, :],
                                     start=True, stop=True)
                    nc.tensor.matmul(Pn2[:, jj, :], XT2[:, sl], w2x[:, l, :],
                                     start=True, stop=True)
                    if not last:
                        nc.tensor.matmul(Pnue[:, jj, :], XT3[:, sl],
                                         w3x[:, l, :], start=True, stop=True)
                t1 = sbE.tile([P, JT // 2, D], f32, tag="t1")
                t2 = sbE.tile([P, JT // 2, D], f32, tag="t2")
                nc.scalar.activation(t1[:], Pn1[:], RELU)
                nc.scalar.activation(t2[:], Pn2[:], RELU)
                if not last:
                    Epad = sbE.tile([P, JT // 2, 2 * D], fp8,
                                    tag=f"epad{hf}")
                    nc.scalar.activation(Epad[:, :, 0:D], Pnue[:], RELU)
                    nc.vector.tensor_tensor(Epad[:, :, D:2 * D],
                                            t1[:], t2[:], ADD)
                    Lg = LgA if hf == 0 else LgB
                    Gg = GgA if hf == 0 else GgB
                    nc.sync.dma_start(out=Lg[:], in_=Epad[:])
                    collective("AllGather", BYPASS, Lg[:], Gg[:], Gg[0])
                else:
                    hs = sbE.tile([P, JT // 2, D], bf16, tag="hs")
                    nc.vector.tensor_tensor(hs[:], t1[:], t2[:], ADD)
                    for jj in range(JT // 2):
                        nc.tensor.matmul(
                            Ppool[:], hs[:, jj, :], ones_mv[:],
                            start=(hf == 0 and jj == 0),
                            stop=(hf == 1 and jj == JT // 2 - 1),
                        )

            for l in range(3):
                last = l == 2
                wue = 2 * D if not last else D
                Pcl0 = psA.tile([D, HC], f32, tag="acc_cl0")
                Pcl1 = psA.tile([D, HC], f32, tag="acc_cl1")
                Pue0 = psA.tile([wue, HC], f32, tag="acc_ue0")
                Pue1 = psA.tile([wue, HC], f32, tag="acc_ue1")
                if last:
                    Ppool = psE.tile([D, 1], f32, tag="pooled")
                else:
                    Ppool = None

                if l == 0:
                    # H0 is local; pass A is paced by the arriving A chunks
                    allp = [2 * kp for kp in range(KT // 2)]
                    acc_mms(0, allp, 0, Pcl0, Pue0, 0, KT - 2)
                    epilogue_half(0, 0, Pcl0, Pue0)
                    acc_mms(0, allp, 1, Pcl1, Pue1, 0, KT - 2)
                    epilogue_half(0, 1, Pcl1, Pue1)
                    keep_warm(NKEEP)
                else:
                    # gathered halves arrive as alpha (coll A), beta (coll B)
                    nc.sync.dma_start(
                        out=Hb4[:, :, 0:JT // 2, :],
                        in_=GgA[:].rearrange("c p j d -> p c j d"))
                    nc.sync.dma_start(
                        out=Hb4[:, :, JT // 2:JT, :],
                        in_=GgB[:].rearrange("c p j d -> p c j d"))
                    acc_mms(l, alpha, 0, Pcl0, Pue0, alpha[0], beta[-1])
                    acc_mms(l, beta, 0, Pcl0, Pue0, alpha[0], beta[-1])
                    epilogue_half(l, 0, Pcl0, Pue0, Ppool)
                    acc_mms(l, alpha, 1, Pcl1, Pue1, alpha[0], beta[-1])
                    acc_mms(l, beta, 1, Pcl1, Pue1, alpha[0], beta[-1])
                    epilogue_half(l, 1, Pcl1, Pue1, Ppool)
                    if not last:
                        keep_warm(NKEEP)

            # ---- pooled vector -> AllReduce -> head MLP ------------------
            pl_s = sbE.tile([D, 1], f32, tag="pl")
            nc.vector.tensor_copy(pl_s[:], Ppool[:])
            nc.sync.dma_start(out=prd_l[:], in_=pl_s[:])
            collective("AllReduce", ADD, prd_l[:], prd_s[:], prd_s[:])
            pvec = sbE.tile([D + 1, 1], f32, tag="pvec")
            zt = sbE.tile([D + 1, 1], f32, tag="zt")
            nc.gpsimd.memset(pvec[D:D + 1, :], 1.0)
            nc.gpsimd.memset(zt[D:D + 1, :], 1.0)
            nc.sync.dma_start(out=pvec[0:D, :], in_=prd_s[:])
            Pz = psE.tile([D, 1], f32, tag="pooled")
            nc.tensor.matmul(Pz[:], q1x[:], pvec[:], start=True, stop=True)
            nc.scalar.activation(zt[0:D, :], Pz[:], RELU)
            Po = psE.tile([1, 1], f32, tag="pooled")
            nc.tensor.matmul(Po[:], q2x[:], zt[:], start=True, stop=True)
            o_s = sbE.tile([1, 1], f32, tag="os")
            nc.vector.tensor_copy(o_s[:], Po[:])
            nc.sync.dma_start(out=out_d[:], in_=o_s[:])

    nc.compile()
    return nc


def _get_module():
    global _CACHED
    if _CACHED is None:
        _CACHED = _build_module()
    return _CACHED


def prep_in_maps(inputs):
    import ml_dtypes

    f = np.float32
    f8 = ml_dtypes.float8_e4m3
    bf = ml_dtypes.bfloat16
    A_cl = np.asarray(inputs["A_cl"], f)
    A_ue = np.asarray(inputs["A_ue"], f)
    ones_row = np.ones((1, N), f)

    Xcat = np.ascontiguousarray(np.vstack([
        np.asarray(inputs["X_cl_1"], f).T,
        np.asarray(inputs["X_cl_2"], f).T,
        np.asarray(inputs["X_ue"], f).T,
        ones_row,
    ]).astype(bf))

    # layer-0 fused block-diagonal weights, output scale SH
    Wcat = np.zeros((7, 3 * D), f)
    Wcat[0:2, 0:D] = np.asarray(inputs["W1_w0"], f) * SH
    Wcat[2:4, D:2 * D] = np.asarray(inputs["W2_w0"], f) * SH
    Wcat[4:6, 2 * D:3 * D] = np.asarray(inputs["W3_w0"], f) * SH
    Wcat[6, 0:D] = np.asarray(inputs["W1_b0"], f) * SH
    Wcat[6, D:2 * D] = np.asarray(inputs["W2_b0"], f) * SH
    Wcat[6, 2 * D:3 * D] = np.asarray(inputs["W3_b0"], f) * SH

    def wx(w, b):
        # [3, D, D] + [3, D] -> [D+1, 3, D]; input X^T carries scale SA*SH,
        # layers 1-2 re-emit H*SH, layer 3 emits unscaled H.
        w = np.asarray(w, f)
        b = np.asarray(b, f)
        cols = []
        for i in range(3):
            w_scale = (1.0 / SA) if i < 2 else (1.0 / (SA * SH))
            b_scale = SH if i < 2 else 1.0
            cols.append(np.vstack([w[i] * w_scale, b[i][None, :] * b_scale]))
        return np.ascontiguousarray(np.stack(cols, axis=1))

    common = {
        "Xcat": Xcat,
        "Wcat": np.ascontiguousarray(Wcat.astype(bf)),
        "w1x": wx(inputs["W1_w"], inputs["W1_b"]).astype(bf),
        "w2x": wx(inputs["W2_w"], inputs["W2_b"]).astype(bf),
        "w3x": wx(inputs["W3_w"], inputs["W3_b"]).astype(bf),
        "q1x": np.ascontiguousarray(
            np.vstack([np.asarray(inputs["Q_w1"], f),
                       np.asarray(inputs["Q_b1"], f)[None, :]])
        ),
        "q2x": np.ascontiguousarray(
            np.vstack([np.asarray(inputs["Q_w2"], f),
                       np.asarray(inputs["Q_b2"], f)[None, :]])
        ),
    }

    # A blocks: [p, h, k, m, r'] = A_m[c*R + h*HC + r', k*P + p] * SA, fp8
    Acl8 = (A_cl * SA).astype(f8)
    Aue8 = (A_ue * SA).astype(f8)

    in_maps = []
    for c in range(M):
        rs = slice(c * R, (c + 1) * R)
        # [R, N] -> [h, r', k, p] -> [p, h, k, r']
        acl = Acl8[rs, :].reshape(2, HC, KT, P).transpose(3, 0, 2, 1)
        aue = Aue8[rs, :].reshape(2, HC, KT, P).transpose(3, 0, 2, 1)
        m = dict(common)
        m["Aall"] = np.ascontiguousarray(
            np.stack([acl, aue], axis=3))  # [P, 2, KT, 2, HC]
        in_maps.append(m)
    return in_maps


def kernel(**inputs):
    global LAST_EXEC_NS, LAST_PROFILE
    nc = _get_module()
    from concourse.bass_utils import run_bass_kernel_spmd

    in_maps = prep_in_maps(inputs)
    res = run_bass_kernel_spmd(nc, in_maps, core_ids=list(range(M)), trace=False)
    LAST_EXEC_NS = res.exec_time_ns
    LAST_PROFILE = res.profile_json
    return np.asarray(res.results[0]["out"], np.float32)
